# revision 75
# baseline (speedup 1.0000x reference)
"""Self-contained Trainium2 kernel for nn_AssemblyArrayComponent_9019431322130.

Data-parallel over batch: 16 samples -> 8 cores x 2 samples.
Host folds (w_in @ conv1 @ bn1) and (conv2 @ bn2) into plain matmuls
(stride==kernel convs are reshapes); device runs the whole net per core:
  GEMM1+gelu -> GEMM2+gelu -> linear attention -> FF -> Mamba-2 SSD (chunked,
  Q=128) -> gated RMS -> out proj -> RMS -> LN.
Activations live as [d, t] (feature on partition, t = 2*512 tokens sample-major).

v3: the SSD decay matrices are built on the PE as fp16 hi/lo outer-product
differences accumulated in PSUM f32 (plus a +M strict-upper mask matmul so
the exp underflows to exactly 0 in the masked region), replacing the
DVE/Pool broadcast-subtract chain and all Sbc DMA broadcasts.  Y keeps two
PSUM accumulators (intra+Dskip vs state) so only the state term is scaled
by cp.  elu+1 is exp->min fused with a relu via scalar_tensor_tensor.
LN2's affine folds into ff1; D_skip folds into a diagonal matmul.
"""
import sys
sys.path.insert(0, '/opt/trn_rl_repo')
import numpy as np
import ml_dtypes

import concourse.bass as bass
import concourse.tile as tile
import concourse.mybir as mybir
from concourse import bacc, library_config
from concourse.bass_utils import run_bass_kernel_spmd

f32 = mybir.dt.float32
bf16 = mybir.dt.bfloat16
f16 = mybir.dt.float16
AF = mybir.ActivationFunctionType
OP = mybir.AluOpType
BF = ml_dtypes.bfloat16

# Guide the act-table chooser: hide exp/ln from their single-function sets so
# ln+exp phases resolve to natural_log_exp_and_others.
import concourse.hw_specs as _hw_specs
from concourse import bacc as _bacc_mod
_ORIG_TABLES = _hw_specs.get_activation_tables


def _patched_tables(arch):
    out = {}
    for name, s in _ORIG_TABLES(arch).items():
        s2 = set(s)
        if name == 'exp_and_others':
            s2.discard(AF.Exp)
        if name == 'natural_log':
            s2.discard(AF.Ln)
        out[name] = s2
    return out


_bacc_mod.get_activation_tables = _patched_tables

B, L, E = 16, 16384, 16
H = 128
NH, DH = 4, 32
FF = 256
D_STATE, HEADDIM = 32, 32
D_INNER = 2 * H
NHEADS = 8
CONV_DIM = 320
DCONV = 4
LC = 512
BN_EPS = 1e-5
Q = 128          # SSD chunk
NCH = 4          # chunks per sample
BLOC = 2         # samples per core
T = BLOC * LC    # 1024 tokens per core
MBIG = 350.0     # strict-upper mask magnitude (underflows exp to 0)


# (name, rows, cols_or_tuple, dtype-class) — "b": bf16, "h": fp16 bits stored
# in the bf16 pack (bitcast at use), "f": f32.  Ordered by first use so the
# weight-pack DMA chunks can interleave with the xT input tiles.
WSPEC = [
    ("wW1", 128, 128, "b"), ("wW2", 128, (4, 128), "b"),
    ("cmat", 128, 128, "b"), ("onesm128", 128, 128, "b"),
    ("wq", 128, 128, "b"), ("wk", 128, 128, "b"), ("wv", 128, 128, "b"),
    ("bkr", 1, 128, "b"), ("bvr", 1, 128, "b"), ("ones1r", 1, 128, "b"),
    ("onecol", 128, 1, "b"), ("sel4T", 128, 4, "b"), ("sel4", 4, 128, "b"),
    ("wo", 128, 128, "b"),
    ("ff1w", 128, 256, "b"), ("ff2w", 128, (2, 128), "b"),
    ("ipwz", 128, 256, "b"), ("ipwd", 128, 8, "b"),
    ("cvw", 128, (12, 128), "b"),
    ("eye", 128, 128, "b"), ("sel8", 8, 256, "b"),
    ("hones2", 16, 1024, "h"), ("nbones8", 8, 512, "h"),
    ("utri", 128, 128, "h"), ("eyeblk", 128, 512, "h"),
    ("ddiag", 128, (2, 128), "b"),
    ("onesm256", 128, 128, "b"), ("cmato", 128, 128, "b"),
    ("outw", 128, (2, 128), "b"),
    ("b1", 128, 1, "f"), ("b2", 128, 1, "f"),
    ("bq", 128, 1, "f"),
    ("olng", 128, 1, "f"), ("olnb", 128, 1, "f"),
    ("bo", 128, 1, "f"),
    ("ff1b", 128, 2, "f"), ("ff2b", 128, 1, "f"),
    ("convb", 128, 3, "f"),
    ("dtbias", 8, 1, "f"), ("A2", 8, 1, "f"),
    ("epsln", 128, 1, "f"), ("epsrms", 128, 1, "f"),
]
W_OFF = {}
WF_COLS = 0
WB_COLS = 0
for _nm, _r, _c, _d in WSPEC:
    _n = int(np.prod(_c)) if isinstance(_c, tuple) else _c
    if _d == "f":
        W_OFF[_nm] = WF_COLS; WF_COLS += _n
    else:
        W_OFF[_nm] = WB_COLS; WB_COLS += _n


def _ap(t_ap, offset_elems, dims):
    return bass.AP(t_ap.tensor, t_ap.offset + offset_elems, dims)


def build_nc():
    nc = bacc.Bacc('TRN2', target_bir_lowering=False, debug=False, num_devices=8)
    dram = {}

    def din(name, shape, dt):
        dram[name] = nc.dram_tensor(name, shape, dt, kind="ExternalInput")
        return dram[name]

    xT = din("xT", [128, 4096], bf16)
    wpackf = din("wpackf", [128, WF_COLS], f32)
    wpackb = din("wpackb", [128, WB_COLS], bf16)
    out_d = nc.dram_tensor("out", [128, 1024], bf16, kind="ExternalOutput")

    with tile.TileContext(nc) as tc:
        with (
            tc.tile_pool(name="wp", bufs=1) as wp,      # weights/consts
            tc.tile_pool(name="ap", bufs=1) as apool,   # persistent activations
            tc.tile_pool(name="tp", bufs=2) as tp,      # transients
            tc.tile_pool(name="pw", bufs=3, space="PSUM") as pw,   # wide psum
            tc.tile_pool(name="pb", bufs=2, space="PSUM") as pb,   # block psum
            tc.tile_pool(name="py", bufs=3, space="PSUM") as py,   # Y accum
        ):
            wpf = wp.tile([128, WF_COLS], f32, tag="wpf")
            wpb = wp.tile([128, WB_COLS], bf16, tag="wpb")
            xTs = apool.tile([128, 4096], bf16, tag="bigB", name="xTs")
            _c1 = W_OFF["ff1w"]          # end of G1/attention weights
            _c2 = W_OFF["cvw"]           # end of FF/in_proj weights
            # xT chunks + wW1 on the sync queue (GEMM1 feed); weight pack on
            # the gpsimd + scalar queues so GEMM1 is never blocked behind it.
            # one queue: HWDGE issues are a global FIFO, so order by need
            nc.sync.dma_start(wpb[:, 0:128], wpackb[:, 0:128])       # wW1
            nc.sync.dma_start(wpf[:], wpackf[:])                     # biases
            nc.sync.dma_start(xTs[:, 0:1024], xT[:, 0:1024])
            nc.sync.dma_start(wpb[:, 128:_c1], wpackb[:, 128:_c1])
            nc.sync.dma_start(xTs[:, 1024:2560], xT[:, 1024:2560])
            nc.sync.dma_start(xTs[:, 2560:4096], xT[:, 2560:4096])
            nc.sync.dma_start(wpb[:, _c1:_c2], wpackb[:, _c1:_c2])
            nc.sync.dma_start(wpb[:, _c2:], wpackb[:, _c2:])
            # per-engine warm-ups: absorb the weight-DMA waits once per engine
            wa0 = tp.tile([1, 4], f32, tag="warm", bufs=1)
            nc.vector.tensor_copy(wa0[:], wpf[0:1, 0:4])
            wb0 = tp.tile([1, 4], bf16, tag="warm", bufs=1)
            nc.vector.tensor_copy(wb0[:], wpb[0:1, 0:4])
            wg = tp.tile([2, 4], f32, tag="warm", bufs=1)
            nc.gpsimd.partition_broadcast(wg[:], wpf[0:1, 0:4])
            W = {"xT": xTs}
            for nm, rows, cols, dt in WSPEC:
                off = W_OFF[nm]
                buf = wpf if dt == "f" else wpb
                ncols = int(np.prod(cols)) if isinstance(cols, tuple) else cols
                apv = buf[0:rows, off:off + ncols]
                if dt == "h":
                    apv = apv.bitcast(f16)
                if isinstance(cols, tuple):
                    apv = apv.rearrange("p (a b) -> p a b", a=cols[0])
                W[nm] = apv

            # ------- GEMM1 + gelu, GEMM2 interleaved per sample -------
            h1 = apool.tile([128, 4096], bf16, tag="bigA", name="h1")
            h_bfs = [apool.tile([128, 512], bf16, tag=f"h_bf{s}",
                                name=f"h_bf{s}") for s in range(BLOC)]
            # all 8 G1 matmuls first (PE is in-order; don't block ready work
            # behind G2 mms that wait on Act gelus)
            g1ps = []
            for i in range(8):
                pool = pw if i % 2 == 0 else py
                ps = pool.tile([128, 512], f32,
                               tag="psw" if i % 2 == 0 else "Yp",
                               name=f"g1ps{i}")
                nc.tensor.matmul(ps[:], W["wW1"][:],
                                 W["xT"][:, bass.ts(i, 512)],
                                 start=True, stop=True)
                g1ps.append(ps)
            g2ps = []
            for grp in range(2):
                for j in range(4):
                    i = 4 * grp + j
                    nc.scalar.activation(h1[:, bass.ts(i, 512)], g1ps[i][:],
                                         AF.Gelu_apprx_tanh,
                                         bias=W["b1"][:, 0:1])
                s = grp
                ps = pw.tile([128, 512], f32, tag="psw", name=f"g2ps{s}")
                for k in range(4):
                    rhs = _ap(h1[:], s * 2048 + k, [list(h1[:].ap[0]), [4, 512]])
                    nc.tensor.matmul(ps[:], W["wW2"][:, k, :], rhs,
                                     start=(k == 0), stop=(k == 3))
                g2ps.append(ps)
                nc.scalar.activation(h_bfs[s][:], g2ps[s][:],
                                     AF.Gelu_apprx_tanh, bias=W["b2"][:, 0:1])

            # ---------------- LayerNorm helper (centered via PE) ----------
            # half-major so sample 0's chain flows without waiting sample 1;
            # sq_act computes the square on Act directly from PSUM (parallel
            # with the xcs copy on DVE).
            def layer_norm(xh, g, b, eps, tagp="ln", cmat="cmat", affine=True,
                           out_dt=bf16, sq_act=False):
                halves = []
                out = None
                if affine:
                    out = apool.tile([128, 1024], out_dt, tag=tagp + "_out")
                xcsl, eql, rstdl, xcl_ps = [], [], [], []
                for hf in range(2):
                    xc = pw.tile([128, 512], f32, tag="psw",
                                 name=f"{tagp}xc{hf}")
                    nc.tensor.matmul(xc[:], W[cmat][:], xh[hf],
                                     start=True, stop=True)
                    xcs = tp.tile([128, 512], bf16, tag="ln_xcs", bufs=2)
                    nc.vector.tensor_copy(xcs[:], xc[:])
                    xcsl.append(xcs)
                    xcl_ps.append(xc)
                for hf in range(2):
                    sq = tp.tile([128, 512], bf16, tag="ln_sq", bufs=2)
                    if sq_act:
                        nc.scalar.activation(sq[:], xcl_ps[hf][:], AF.Square)
                    else:
                        nc.vector.tensor_tensor(out=sq[:], in0=xcsl[hf][:],
                                                in1=xcsl[hf][:], op=OP.mult)
                    eq = pw.tile([128, 512], f32, tag="psw",
                                 name=f"{tagp}eq{hf}")
                    nc.tensor.matmul(eq[:], W["onesm128"][:], sq[:],
                                     start=True, stop=True)
                    eql.append(eq)
                for hf in range(2):
                    lnv = tp.tile([128, 512], f32, tag="ln_lnv", bufs=2)
                    nc.scalar.activation(lnv[:], eql[hf][:], AF.Ln,
                                         bias=eps[:, 0:1])
                    rstd = tp.tile([128, 512], bf16, tag="ln_rstd", bufs=3)
                    nc.scalar.activation(rstd[:], lnv[:], AF.Exp, scale=-0.5)
                    rstdl.append(rstd)
                for hf in range(2):
                    t2 = tp.tile([128, 512], bf16, tag=tagp + "_t2", bufs=2)
                    nc.vector.tensor_tensor(out=t2[:], in0=xcsl[hf][:],
                                            in1=rstdl[hf][:], op=OP.mult)
                    if affine:
                        nc.vector.tensor_scalar(out=out[:, bass.ts(hf, 512)],
                                                in0=t2[:], scalar1=g[:, 0:1],
                                                scalar2=b[:, 0:1],
                                                op0=OP.mult, op1=OP.add)
                    halves.append(t2)
                return out, halves

            # ---------------- attention (ln1 affine folded into wq/wk/wv) --
            _, a_hv = layer_norm([h_bfs[0][:], h_bfs[1][:]],
                                 None, None, W["epsln"], tagp="ln1",
                                 affine=False)

            # q in [dq, t]: elu+1 = min(exp(x),1) + max(x,0)
            q_bf = apool.tile([128, 1024], bf16, tag="q_bf")
            em_q = apool.tile([128, 1024], bf16, tag="em_q")
            xr_q = apool.tile([128, 1024], bf16, tag="xr_q")
            meK = apool.tile([128, 8, 128], bf16, tag="meK")
            vT = apool.tile([128, 8, 128], bf16, tag="vT")
            em_k = apool.tile([128, 1024], bf16, tag="em_k")
            xr_k = apool.tile([128, 8, 128], bf16, tag="xr_k")
            for hf in range(2):
                ps = pw.tile([128, 512], f32, tag="psw", name=f"qps{hf}")
                nc.tensor.matmul(ps[:], W["wq"][:], a_hv[hf][:],
                                 start=True, stop=True)
                psk = pw.tile([128, 512], f32, tag="psw", name=f"psk{hf}")
                psv = pw.tile([128, 512], f32, tag="psw", name=f"psv{hf}")
                for q4 in range(4):
                    nc.tensor.matmul(psk[:, bass.ts(q4, 128)],
                                     a_hv[hf][:, bass.ts(q4, 128)], W["wk"][:],
                                     start=True, stop=False)
                    nc.tensor.matmul(psk[:, bass.ts(q4, 128)],
                                     W["ones1r"][0:1, :], W["bkr"][0:1, :],
                                     start=False, stop=True)
                    nc.tensor.matmul(psv[:, bass.ts(q4, 128)],
                                     a_hv[hf][:, bass.ts(q4, 128)], W["wv"][:],
                                     start=True, stop=False)
                    nc.tensor.matmul(psv[:, bass.ts(q4, 128)],
                                     W["ones1r"][0:1, :], W["bvr"][0:1, :],
                                     start=False, stop=True)
                nc.scalar.activation(em_k[:, bass.ts(hf, 512)],
                                     psk[:], AF.Exp)
                nc.vector.tensor_scalar(
                    out=xr_k[:].rearrange("p a b -> p (a b)")[:, bass.ts(hf, 512)],
                    in0=psk[:], scalar1=0.0,
                    scalar2=None, op0=OP.max)
                if hf == 0:
                    nc.scalar.copy(
                        vT[:].rearrange("p a b -> p (a b)")[:, 0:512], psv[:])
                else:
                    nc.vector.tensor_copy(
                        vT[:].rearrange("p a b -> p (a b)")[:, 512:1024],
                        psv[:])
                nc.vector.tensor_scalar(
                    out=meK[:].rearrange("p a b -> p (a b)")[:, bass.ts(hf, 512)],
                    in0=em_k[:, bass.ts(hf, 512)], scalar1=1.0,
                    scalar2=None, op0=OP.min)
                nc.scalar.activation(em_q[:, bass.ts(hf, 512)], ps[:],
                                     AF.Exp, bias=W["bq"][:, 0:1])
                nc.vector.tensor_scalar(out=xr_q[:, bass.ts(hf, 512)],
                                        in0=ps[:], scalar1=W["bq"][:, 0:1],
                                        scalar2=0.0, op0=OP.add, op1=OP.max)
                nc.vector.scalar_tensor_tensor(
                    out=q_bf[:, bass.ts(hf, 512)],
                    in0=em_q[:, bass.ts(hf, 512)], scalar=1.0,
                    in1=xr_q[:, bass.ts(hf, 512)], op0=OP.min, op1=OP.add)

            # kv[d,e] per (b,h) stacked on partitions; ksum via ones rhs
            kv_sb, ksumM = [], []
            for s in range(BLOC):
                kvp = pb.tile([128, 32], f32, tag="psb")
                for hh in range(4):
                    for tt in range(4):
                        nc.tensor.matmul(
                            kvp[32 * hh:32 * hh + 32, :],
                            xr_k[:, 4 * s + tt, 32 * hh:32 * hh + 32],
                            vT[:, 4 * s + tt, 32 * hh:32 * hh + 32],
                            start=(tt == 0), stop=False,
                            tile_position=(0, 32 * hh))
                    for tt in range(4):
                        nc.tensor.matmul(
                            kvp[32 * hh:32 * hh + 32, :],
                            meK[:, 4 * s + tt, 32 * hh:32 * hh + 32],
                            vT[:, 4 * s + tt, 32 * hh:32 * hh + 32],
                            start=False, stop=(tt == 3),
                            tile_position=(0, 32 * hh))
                kv = apool.tile([128, 32], bf16, tag=f"kv{s}")
                nc.scalar.copy(kv[:], kvp[:])
                kv_sb.append(kv)
                ksp = pb.tile([128, 1], f32, tag="psb")
                for tt in range(4):
                    nc.tensor.matmul(ksp[:], xr_k[:, 4 * s + tt, :],
                                     W["onecol"][:],
                                     start=(tt == 0), stop=False)
                for tt in range(4):
                    nc.tensor.matmul(ksp[:], meK[:, 4 * s + tt, :],
                                     W["onecol"][:],
                                     start=False, stop=(tt == 3))
                km = apool.tile([128, 4], bf16, tag=f"ksumM{s}")
                ksp_bc = _ap(ksp[:], 0, [list(ksp[:].ap[0]), [0, 4]])
                nc.vector.tensor_tensor(out=km[:], in0=ksp_bc,
                                        in1=W["sel4T"][:], op=OP.mult)
                ksumM.append(km)

            attnf = apool.tile([128, 1024], bf16, tag="attnf")
            zrbs, zrxss, atps = {}, {}, {}
            for s in range(BLOC):
                den = pb.tile([4, 512], f32, tag="psb", name=f"den{s}")
                nc.tensor.matmul(den[:], ksumM[s][:], q_bf[:, bass.ts(s, 512)],
                                 start=True, stop=True)
                zrb = tp.tile([4, 512], bf16, tag="zrb")
                with nc.allow_low_precision(reason="z feeds bf16 matmul rhs"):
                    nc.vector.reciprocal(zrb[:], den[:])
                zrbs[s] = zrb
            for s in range(BLOC):
                atp = pw.tile([128, 512], f32, tag="psw", name=f"atp{s}")
                for hh in range(4):
                    nc.tensor.matmul(atp[32 * hh:32 * hh + 32, :],
                                     kv_sb[s][32 * hh:32 * hh + 32, :],
                                     q_bf[32 * hh:32 * hh + 32, bass.ts(s, 512)],
                                     start=True, stop=True,
                                     tile_position=(32 * hh, 32 * hh))
                atps_sb = tp.tile([128, 512], bf16, tag="atps", bufs=2)
                nc.scalar.copy(atps_sb[:], atp[:])
                atps[s] = atps_sb
            for s in range(BLOC):
                zrx = pb.tile([128, 512], f32, tag="psb", name=f"zrx{s}")
                nc.tensor.matmul(zrx[:], W["sel4"][:], zrbs[s][:],
                                 start=True, stop=True)
                nc.vector.tensor_tensor(out=attnf[:, bass.ts(s, 512)],
                                        in0=atps[s][:],
                                        in1=zrx[:], op=OP.mult)

            h2_bf = apool.tile([128, 1024], bf16, tag="h2_bf")
            h2ps = []
            for hf in range(2):
                ps = pw.tile([128, 512], f32, tag="psw", name=f"h2ps{hf}")
                nc.tensor.matmul(ps[:], W["wo"][:], attnf[:, bass.ts(hf, 512)],
                                 start=True, stop=True)
                h2ps.append(ps)
            for hf in range(2):
                nc.vector.scalar_tensor_tensor(
                    out=h2_bf[:, bass.ts(hf, 512)], in0=h2ps[hf][:],
                    scalar=W["bo"][:, 0:1], in1=h_bfs[hf][:],
                    op0=OP.add, op1=OP.add)

            # ---------------- FF (ln2 affine folded into ff1) -------------
            _, f_hv = layer_norm([h2_bf[:, 0:512], h2_bf[:, 512:1024]],
                                 None, None, W["epsln"], tagp="ln2",
                                 affine=False)
            gff = apool.tile([128, 2, 1024], bf16, tag="bigA", name="gff")
            ffps = {}
            for mt in range(2):
                for hf in range(2):
                    ps = pw.tile([128, 512], f32, tag="psw",
                                 name=f"ffps{mt}{hf}")
                    nc.tensor.matmul(ps[:], W["ff1w"][:, bass.ts(mt, 128)],
                                     f_hv[hf][:],
                                     start=True, stop=True)
                    ffps[(mt, hf)] = ps
            for mt in range(2):
                for hf in range(2):
                    nc.scalar.activation(gff[:, mt, bass.ts(hf, 512)],
                                         ffps[(mt, hf)][:],
                                         AF.Gelu_apprx_tanh,
                                         bias=W["ff1b"][:, mt:mt + 1])
            # h3 in a causally-padded layout [128, 2, 515] (3 zero cols per
            # sample) so the fused conv taps can read shifted windows
            h3p = apool.tile([128, 2, 515], bf16, tag="h3p", name="h3p")
            for hf in range(2):
                nc.gpsimd.memset(h3p[:, hf, 0:3], 0.0)
            h3ps = []
            for hf in range(2):
                ps = pw.tile([128, 512], f32, tag="psw", name=f"h3ps{hf}")
                for kt in range(2):
                    nc.tensor.matmul(ps[:], W["ff2w"][:, kt, :],
                                     gff[:, kt, bass.ts(hf, 512)],
                                     start=(kt == 0), stop=(kt == 1))
                h3ps.append(ps)
            for hf in range(2):
                nc.vector.scalar_tensor_tensor(
                    out=h3p[:, hf, 3:515], in0=h3ps[hf][:],
                    scalar=W["ff2b"][:, 0:1], in1=h2_bf[:, bass.ts(hf, 512)],
                    op0=OP.add, op1=OP.add)

            # ---------------- Mamba: in_proj ----------------
            # dt first (critical path); the depthwise conv is folded into
            # the in_proj weights (taps read the zero-padded h3 directly)
            dtraw = apool.tile([8, 1024], f32, tag="dtraw")
            dps = []
            for hf in range(2):
                ps = pw.tile([8, 512], f32, tag="psw", name=f"dps{hf}")
                nc.tensor.matmul(ps[:], W["ipwd"][:],
                                 h3p[:, hf, 3:515], start=True,
                                 stop=True)
                dps.append(ps)
            for hf in range(2):
                nc.scalar.copy(dtraw[:, bass.ts(hf, 512)],
                               dps[hf][0:8, :])

            # softplus(dtraw + dt_bias) = ln(1 + exp(x)); per-sample so the
            # s0 SSD stream launches before s1 finishes
            dt2a = apool.tile([8, 1024], f32, tag="dt2a")
            dt2bf = apool.tile([8, 1024], bf16, tag="dt2bf")
            S2 = apool.tile([8, 1024], f32, tag="S2")
            S2h = apool.tile([8, 1024], f16, tag="S2h")
            S2l = apool.tile([8, 1024], f16, tag="S2l")
            S2hl_d = nc.dram_tensor("S2hl", [16, 1024], f16)
            S2HL = apool.tile([16, 1024], f16, tag="S2HL")
            S2T4 = [apool.tile([8, 1024], f16, tag=f"S2T4_{s}",
                               name=f"S2T4_{s}")
                    for s in range(BLOC)]
            for s in range(BLOC):
                sl = bass.ts(s, 512)
                espx = tp.tile([8, 512], f32, tag="spbuf", bufs=2)
                nc.scalar.activation(espx[:], dtraw[:, sl], AF.Exp,
                                     bias=W["dtbias"][0:8, 0:1])
                nc.scalar.activation(dt2a[:, sl], espx[:], AF.Ln, bias=1.0)
                nc.gpsimd.tensor_copy(dt2bf[:, sl], dt2a[:, sl])
                for g in range(4):
                    gg = 4 * s + g
                    nc.vector.tensor_tensor_scan(
                        out=S2[:, bass.ts(gg, 128)],
                        data0=dt2a[:, bass.ts(gg, 128)],
                        data1=dt2a[:, bass.ts(gg, 128)], initial=0.0,
                        op0=OP.add, op1=OP.bypass)
                nc.vector.tensor_copy(S2h[:, sl], S2[:, sl])
                nc.vector.tensor_tensor(out=S2l[:, sl], in0=S2[:, sl],
                                        in1=S2h[:, sl], op=OP.subtract)
                nc.sync.dma_start(S2hl_d[0:8, s * 512:(s + 1) * 512],
                                  S2h[:, sl])
                nc.gpsimd.dma_start(S2hl_d[8:16, s * 512:(s + 1) * 512],
                                    S2l[:, sl])
                nc.sync.dma_start(S2HL[:, sl],
                                  S2hl_d[:, s * 512:(s + 1) * 512])
                eng = nc.gpsimd if s == 0 else nc.sync
                for hl in range(2):
                    srcp = bass.AP(S2hl_d[:].tensor, hl * 8 * 1024 + s * 512,
                                   [[128, 4], [1024, 8], [1, 128]])
                    eng.dma_start(S2T4[s][4 * hl:4 * hl + 4, :], srcp)

            # fused in_proj+depthwise-causal-conv on PE: tap k reads the
            # padded h3 shifted by k, lhsT = ipw_x-slice * diag(conv_w[:,k])
            xbcs = apool.tile([128, 6, 512], bf16, tag="xbcs")
            zgs = apool.tile([128, 2, 1024], bf16, tag="bigB", name="zgs")
            for s in range(BLOC):
                cvps = []
                for ct in range(3):
                    rows = 128 if ct < 2 else 64
                    pool_ = pw if s == 0 else py
                    cps_ = pool_.tile([128, 512], f32,
                                      tag="psw" if s == 0 else "Yp",
                                      name=f"cvps{s}{ct}")
                    for k in range(4):
                        nc.tensor.matmul(
                            cps_[0:rows, :],
                            W["cvw"][:, 4 * ct + k, 0:rows],
                            h3p[:, s, k:512 + k],
                            start=(k == 0), stop=(k == 3))
                    cvps.append(cps_)
                for ct in range(3):
                    rows = 128 if ct < 2 else 64
                    nc.scalar.activation(xbcs[0:rows, 3 * s + ct, :],
                                         cvps[ct][0:rows, :], AF.Silu,
                                         bias=W["convb"][0:rows, ct:ct + 1])
                # z-gate pair for this half between the conv silus so the
                # zps PSUM slots recycle before the s1 z matmuls issue
                mt = s
                zps = []
                for hf in range(2):
                    ps = pw.tile([128, 512], f32, tag="psw",
                                 name=f"zps{mt}{hf}")
                    nc.tensor.matmul(ps[:], W["ipwz"][:, bass.ts(mt, 128)],
                                     h3p[:, hf, 3:515],
                                     start=True, stop=True)
                    zps.append(ps)
                for hf in range(2):
                    nc.scalar.activation(zgs[:, mt, bass.ts(hf, 512)],
                                         zps[hf][:], AF.Silu)

            # cp = exp(A*S2); wend = exp(A*(S_end - S2)) — after the silu
            # phase (the dr/LT stream does not need them; only the state
            # chain does), so the conv PSUM slots recycle sooner
            cp8 = apool.tile([8, 1024], bf16, tag="cp8")
            wend_bf = apool.tile([8, 1024], bf16, tag="wend_bf")
            for s in range(BLOC):
                sl = bass.ts(s, 512)
                nc.scalar.activation(cp8[:, sl], S2[:, sl], AF.Exp,
                                     scale=W["A2"][0:8, 0:1])
                wl = tp.tile([8, 512], f32, tag="wl", bufs=2)
                send_ap = _ap(S2[:], s * 512 + 127,
                              [list(S2[:].ap[0]), [128, 4], [0, 128]])
                nc.vector.tensor_tensor(
                    out=wl[:].rearrange("p (c j) -> p c j", c=4),
                    in0=send_ap,
                    in1=S2[:, sl].rearrange("p (c j) -> p c j", c=4),
                    op=OP.subtract)
                nc.scalar.activation(wend_bf[:, sl], wl[:], AF.Exp,
                                     scale=W["A2"][0:8, 0:1])
            # wendT [128, 8*8] (cols g*8+h); 4 transposes per PSUM + 1 copy
            wendT = apool.tile([128, 64], bf16, tag="wendT")
            for gb in range(2):
                ptw = pb.tile([128, 32], bf16, tag="psb", name=f"ptw{gb}")
                for k in range(4):
                    g = 4 * gb + k
                    nc.tensor.transpose(ptw[:, bass.ts(k, 8)],
                                        wend_bf[:, bass.ts(g, 128)],
                                        W["eye"][0:8, 0:8])
                nc.vector.tensor_copy(wendT[:, bass.ts(gb, 32)], ptw[:])

            # dt broadcast + xdt
            xdt = apool.tile([128, 6, 512], bf16, tag="bigD", name="xdt")
            dxs = {}
            for s in range(BLOC):
                for jt in range(2):
                    dx = pw.tile([128, 512], f32, tag="psw", name=f"dx{s}{jt}")
                    nc.tensor.matmul(dx[:], W["sel8"][:, bass.ts(jt, 128)],
                                     dt2bf[:, bass.ts(s, 512)],
                                     start=True, stop=True)
                    dxs[(s, jt)] = dx
            for s in range(BLOC):
                for jt in range(2):
                    nc.vector.tensor_tensor(out=xdt[:, 3 * s + jt, :],
                                            in0=xbcs[:, 3 * s + jt, :],
                                            in1=dxs[(s, jt)][:], op=OP.mult)

            # xdtT [t_local, (tb, ch256)] per sample: batched transposes
            xdtT = [apool.tile([128, 4, 256], bf16, tag=f"xdtT{s}", name=f"xdtT{s}")
                    for s in range(BLOC)]
            for s in range(BLOC):
                ptps = []
                for jt in range(2):
                    ptp = pb.tile([128, 512], bf16, tag="psb",
                                  name=f"ptp{s}{jt}")
                    for tb in range(4):
                        nc.tensor.transpose(
                            ptp[:, bass.ts(tb, 128)],
                            xdt[:, 3 * s + jt, bass.ts(tb, 128)], W["eye"][:])
                    ptps.append(ptp)
                for jt in range(2):
                    out_ap = _ap(xdtT[s][:], jt * 128,
                                 [list(xdtT[s][:].ap[0]), [256, 4], [1, 128]])
                    if s == 1:
                        nc.scalar.copy(
                            out_ap,
                            ptps[jt][:].rearrange("p (a b) -> p a b", a=4))
                    else:
                        nc.vector.tensor_copy(
                            out_ap,
                            ptps[jt][:].rearrange("p (a b) -> p a b", a=4))

            # xdtw = xdtT * wend (per-head, free-broadcast over p)
            xdtw = [apool.tile([128, 4, 256], bf16, tag=f"xdtw{s}", name=f"xdtw{s}")
                    for s in range(BLOC)]
            for s in range(BLOC):
                for tb in range(4):
                    wap = _ap(wendT[:], (4 * s + tb) * 8,
                              [list(wendT[:].ap[0]), [1, 8], [0, 32]])
                    eng = nc.gpsimd if tb % 2 else nc.vector
                    eng.tensor_tensor(
                        out=xdtw[s][:, tb, :].rearrange("p (h q) -> p h q", h=8),
                        in0=xdtT[s][:, tb, :].rearrange("p (h q) -> p h q", h=8),
                        in1=wap, op=OP.mult)

            # BT [t_local, (tb, n32)] per sample
            BT = [apool.tile([128, 4, 32], bf16, tag=f"BT{s}", name=f"BT{s}")
                  for s in range(BLOC)]
            for s in range(BLOC):
                for tbp in range(2):
                    pts = []
                    for k in range(2):
                        tb = 2 * tbp + k
                        pt = pb.tile([128, 128], bf16, tag="psb",
                                     name=f"btp{s}{tb}")
                        nc.tensor.transpose(
                            pt[:], xbcs[:, 3 * s + 2, bass.ts(tb, 128)],
                            W["eye"][:])
                        pts.append(pt)
                    for k in range(2):
                        tb = 2 * tbp + k
                        if tb < 3:
                            nc.vector.tensor_copy(BT[s][:, tb, :],
                                                  pts[k][:, 0:32])

            # chunk states first: the recurrence (DVE) runs while the PE
            # builds the decay matrices.  states[s][c] = state at START of
            # chunk c (bf16 [32, 256]).
            states = [[None] * NCH for _ in range(BLOC)]
            aendSB = {}
            for s in range(BLOC):
                st0 = apool.tile([32, 256], bf16, tag=f"st_{s}_0")
                nc.vector.memset(st0[:], 0.0)
                states[s][0] = st0
            for c in range(NCH - 1):
                for s in range(BLOC):
                    aend8 = tp.tile([8, 32], bf16, tag="aend8", bufs=2)
                    aap = _ap(cp8[:], s * 512 + c * 128 + 127,
                              [list(cp8[:].ap[0]), [0, 32]])
                    nc.vector.tensor_copy(aend8[:], aap)
                    aendB = pb.tile([32, 256], f32, tag="psb",
                                    name=f"aendB{s}{c}")
                    nc.tensor.matmul(aendB[:], aend8[:], W["sel8"][:],
                                     start=True, stop=True,
                                     tile_position=(0, 0))
                    aendSB[(s, c)] = aendB
                for s in range(BLOC):
                    Tp = pb.tile([32, 256], f32, tag="psb", name=f"Tp{s}{c}")
                    nc.tensor.matmul(Tp[:], BT[s][:, c, :], xdtw[s][:, c, :],
                                     start=True, stop=True)
                    st_tmp = tp.tile([32, 256], bf16, tag="st_tmp", bufs=2)
                    nc.vector.tensor_tensor(out=st_tmp[:],
                                            in0=states[s][c][:],
                                            in1=aendSB[(s, c)][:], op=OP.mult)
                    st2 = apool.tile([32, 256], bf16, tag=f"st_{s}_{c + 1}")
                    nc.vector.tensor_tensor(out=st2[:], in0=st_tmp[:],
                                            in1=Tp[:], op=OP.add)
                    states[s][c + 1] = st2

            # Bm/Cm at partition base 0 [32, 512] per sample
            Bm_sb = [apool.tile([32, 512], bf16, tag=f"Bm{s}", name=f"Bm{s}")
                     for s in range(BLOC)]
            Cm_sb = [apool.tile([32, 512], bf16, tag=f"Cm{s}", name=f"Cm{s}")
                     for s in range(BLOC)]
            for s in range(BLOC):
                nc.gpsimd.tensor_copy(Bm_sb[s][:], xbcs[0:32, 3 * s + 2, :])
                nc.gpsimd.tensor_copy(Cm_sb[s][:], xbcs[32:64, 3 * s + 2, :])

            # G = Bm^T Cm per chunk, unmasked (upper region killed by LT=0)
            GTms = [apool.tile([128, 512], bf16, tag=f"GTms{s}",
                               name=f"GTms{s}") for s in range(BLOC)]
            for s in range(BLOC):
                gp = pw.tile([128, 512], f32, tag="psw", name=f"gp{s}")
                for c in range(4):
                    nc.tensor.matmul(gp[:, bass.ts(c, 128)],
                                     Bm_sb[s][:, bass.ts(c, 128)],
                                     Cm_sb[s][:, bass.ts(c, 128)],
                                     start=True, stop=True)
                nc.vector.tensor_copy(GTms[s][:], gp[:])

            # ---- decay matrices via PE outer products (fp16 hi/lo) ----
            # dr[t',(c,j)] = S2_j - S2_t' + M*(j<t');  LT = exp(A_h * dr);
            # MT = LT * G.  Emitted interleaved with the Yp phases and the
            # per-sample tails so each sample drains while the next builds.
            MT = [[apool.tile([128, 512], bf16, tag=f"MT{s}_{hh}",
                              name=f"MT{s}_{hh}")
                   for hh in range(NHEADS)] for s in range(BLOC)]
            y2 = apool.tile([128, 6, 512], bf16, tag="bigD", name="y2")
            yn = apool.tile([128, 6, 512], bf16, tag="bigC", name="yn")
            yfin = apool.tile([128, 1024], bf16, tag="yfin")
            yzs, eqs, rstds, cpxs = {}, {}, {}, {}

            def emit_MT(s, hh):
                dr = pw.tile([128, 512], f32, tag="psw", name=f"dr{s}{hh}")
                # term1: +S2_j broadcast down columns (hi+lo stacked)
                nc.tensor.matmul(dr[:], W["hones2"][:, bass.ts(hh, 128)],
                                 S2HL[:, bass.ts(s, 512)],
                                 start=True, stop=False)
                # term2: -S2_t' per chunk (hi+lo stacked on (hl,c))
                nc.tensor.matmul(dr[:], S2T4[s][:, bass.ts(hh, 128)],
                                 W["nbones8"][:],
                                 start=False, stop=False)
                # +M on the strict upper triangle of each chunk block
                nc.tensor.matmul(dr[:], W["utri"][:], W["eyeblk"][:],
                                 start=False, stop=True)
                LT = tp.tile([128, 512], bf16, tag="LT", bufs=3)
                nc.scalar.activation(LT[:], dr[:], AF.Exp,
                                     scale=-float(hh + 1))
                eng = nc.vector if s == 0 else nc.gpsimd
                eng.tensor_tensor(out=MT[s][hh][:], in0=LT[:],
                                  in1=GTms[s][:], op=OP.mult)

            def emit_Yp(s, jt):
                # YpI = intra + D_skip*xs;  YpS = state terms.
                # y2 = (YpS * cp) + YpI; then the gated-RMS square stage.
                cx = pw.tile([128, 512], f32, tag="psw", name=f"cx{s}{jt}")
                nc.tensor.matmul(cx[:], W["sel8"][:, bass.ts(jt, 128)],
                                 cp8[:, bass.ts(s, 512)],
                                 start=True, stop=True)
                cxs = tp.tile([128, 512], bf16, tag="cpx_sb", bufs=2)
                nc.scalar.copy(cxs[:], cx[:])
                cpxs[(s, jt)] = cxs
                YpI = py.tile([128, 512], f32, tag="Yp", name=f"YpI{s}{jt}")
                YpS = py.tile([128, 512], f32, tag="Yp", name=f"YpS{s}{jt}")
                # chunk 0 has no state yet: zero-fill via 0-weight matmul
                # (hones2 row 0 is zero past col 128)
                nc.tensor.matmul(YpS[:, 0:128],
                                 W["hones2"][0:1, 128:256],
                                 W["hones2"][0:1, 0:128],
                                 start=True, stop=True)
                for c in range(NCH):
                    for hq in range(4):
                        hh = 4 * jt + hq
                        nc.tensor.matmul(
                            YpI[32 * hq:32 * hq + 32, bass.ts(c, 128)],
                            xdtT[s][:, c, 32 * hh:32 * hh + 32],
                            MT[s][hh][:, bass.ts(c, 128)],
                            start=True, stop=False,
                            tile_position=(0, 32 * hq))
                        if c > 0:
                            nc.tensor.matmul(
                                YpS[32 * hq:32 * hq + 32, bass.ts(c, 128)],
                                states[s][c][:, 32 * hh:32 * hh + 32],
                                Cm_sb[s][:, bass.ts(c, 128)],
                                start=True, stop=True,
                                tile_position=(0, 32 * hq))
                # D_skip fold: YpI += Ddiag @ xs
                nc.tensor.matmul(YpI[:], W["ddiag"][:, jt, :],
                                 xbcs[:, 3 * s + jt, :],
                                 start=False, stop=True)
                t1 = tp.tile([128, 512], bf16, tag="t1", bufs=2)
                nc.vector.tensor_tensor(out=t1[:], in0=YpS[:],
                                        in1=cpxs[(s, jt)][:], op=OP.mult)
                nc.vector.tensor_tensor(out=y2[:, 3 * s + jt, :],
                                        in0=t1[:], in1=YpI[:], op=OP.add)
                yz = tp.tile([128, 512], bf16, tag="yz", name=f"yz{s}{jt}")
                nc.vector.tensor_tensor(out=yz[:], in0=y2[:, 3 * s + jt, :],
                                        in1=zgs[:, jt, bass.ts(s, 512)],
                                        op=OP.mult)
                yzs[(s, jt)] = yz
                sqz = tp.tile([128, 512], bf16, tag="sqz", name=f"sqz{s}{jt}")
                nc.vector.tensor_tensor(out=sqz[:], in0=yz[:],
                                        in1=yz[:], op=OP.mult)
                if jt == 0:
                    eqs[s] = pb.tile([128, 512], f32, tag="psb",
                                     name=f"eqmn{s}")
                nc.tensor.matmul(eqs[s][:], W["onesm256"][:], sqz[:],
                                 start=(jt == 0), stop=(jt == 1))

            def emit_mnorm(s, mh):
                # half-split (256 cols) so the out-proj quarters start sooner
                sl = slice(mh * 256, mh * 256 + 256)
                lnv = tp.tile([128, 256], f32, tag="mn_lnv", bufs=2)
                nc.scalar.activation(lnv[:], eqs[s][:, sl], AF.Ln,
                                     bias=W["epsrms"][:, 0:1])
                rstd = tp.tile([128, 256], bf16, tag="mn_rstd", bufs=2)
                nc.scalar.activation(rstd[:], lnv[:], AF.Exp, scale=-0.5)
                # mnorm_w folded into out_w's contraction dim (host-side)
                for jt in range(2):
                    nc.vector.tensor_tensor(
                        out=yn[:, 3 * s + jt, sl], in0=yzs[(s, jt)][:, sl],
                        in1=rstd[:], op=OP.mult)

            def emit_oln(s, hq):
                # out-proj + residual + rms_w + oln centering fused on PE:
                # xc = cmato @ (outw @ yn + h3); quarter-width for latency
                xc = pw.tile([128, 256], f32, tag="psw", name=f"xcol{s}{hq}")
                for kt in range(2):
                    nc.tensor.matmul(xc[:], W["outw"][:, kt, :],
                                     _ap(yn[:], (3 * s + kt) * 512 + hq * 256,
                                         [list(yn[:].ap[0]), [1, 256]]),
                                     start=(kt == 0), stop=False)
                nc.tensor.matmul(xc[:], W["cmato"][:],
                                 h3p[:, s, 3 + hq * 256:3 + hq * 256 + 256],
                                 start=False, stop=True)
                xcs = tp.tile([128, 256], bf16, tag="ol_xcs", bufs=3)
                nc.vector.tensor_copy(xcs[:], xc[:])
                sq = tp.tile([128, 256], bf16, tag="ol_sq", bufs=3)
                nc.vector.tensor_tensor(out=sq[:], in0=xcs[:],
                                        in1=xcs[:], op=OP.mult)
                eq = pw.tile([128, 256], f32, tag="psw", name=f"eqol{s}{hq}")
                nc.tensor.matmul(eq[:], W["onesm128"][:], sq[:],
                                 start=True, stop=True)
                lnv = tp.tile([128, 256], f32, tag="ol_lnv", bufs=3)
                nc.scalar.activation(lnv[:], eq[:], AF.Ln,
                                     bias=W["epsln"][:, 0:1])
                rstd = tp.tile([128, 256], bf16, tag="ol_rstd", bufs=4)
                nc.scalar.activation(rstd[:], lnv[:], AF.Exp, scale=-0.5)
                t2 = tp.tile([128, 256], bf16, tag="oln_t2", bufs=3)
                nc.vector.tensor_tensor(out=t2[:], in0=xcs[:],
                                        in1=rstd[:], op=OP.mult)
                off = s * 512 + hq * 256
                nc.vector.tensor_scalar(out=yfin[:, off:off + 256],
                                        in0=t2[:], scalar1=W["olng"][:, 0:1],
                                        scalar2=W["olnb"][:, 0:1],
                                        op0=OP.mult, op1=OP.add)
                nc.sync.dma_start(out_d[:, off:off + 256],
                                  yfin[:, off:off + 256])

            for hh in range(NHEADS):
                emit_MT(0, hh)
            emit_Yp(0, 0)
            for hh in range(4):
                emit_MT(1, hh)
            emit_Yp(0, 1)
            emit_mnorm(0, 0)
            emit_mnorm(0, 1)
            for hh in range(4, NHEADS):
                emit_MT(1, hh)
            emit_Yp(1, 0)
            emit_Yp(1, 1)
            emit_oln(0, 0)
            emit_oln(0, 1)
            emit_mnorm(1, 0)
            emit_oln(1, 0)
            emit_mnorm(1, 1)
            emit_oln(1, 1)

    nc.compile()
    return nc


# ---------------- host side ----------------
_CACHE = {}


def _f16_as_bf(v):
    """fp16 bit pattern viewed as bfloat16 for the bf16 weight pack."""
    return np.ascontiguousarray(v.astype(np.float16)).view(np.uint16).view(BF)


def _prep(inputs):
    d = {k: np.asarray(v, np.float32) for k, v in inputs.items()}
    inv = 1.0 / np.sqrt(1.0 + BN_EPS)
    W1 = np.einsum('ei,oik->keo', d['w_in'], d['conv1_w']).reshape(128, H)
    b1v = np.einsum('i,oik->o', d['b_in'], d['conv1_w'])
    s1 = d['bn1_g'] * inv
    W1 = W1 * s1[None, :]
    b1v = b1v * s1 + d['bn1_b']
    W2 = np.transpose(d['conv2_w'], (2, 1, 0)) * (d['bn2_g'] * inv)[None, None, :]
    W2sb = np.ascontiguousarray(np.transpose(W2, (1, 0, 2)))          # [i,k,o]
    # ln1 affine folded into wq/wk/wv
    wq_f = d['ln1_g'][:, None] * d['wq']
    wk_f = d['ln1_g'][:, None] * d['wk']
    wv_f = d['ln1_g'][:, None] * d['wv']
    bq_f = d['ln1_b'] @ d['wq']
    bk_f = d['ln1_b'] @ d['wk']
    bv_f = d['ln1_b'] @ d['wv']
    # ln2 affine folded into ff1
    ff1w_f = d['ln2_g'][:, None] * d['ff1_w']
    ff1b_f = d['ff1_b'] + d['ln2_b'] @ d['ff1_w']
    ff2sb = np.ascontiguousarray(d['ff2_w'].reshape(2, 128, 128).transpose(1, 0, 2))
    _cm = (np.eye(128) - np.full((128, 128), 1.0 / 128)) @ np.diag(d['rms_w'])
    _ow2 = (d['mnorm_w'][:, None] * d['out_w']) @ _cm
    outsb = np.ascontiguousarray(_ow2.reshape(2, 128, 128).transpose(1, 0, 2))
    cb = np.zeros((128, 3), np.float32)
    # conv folded into in_proj: cvw[:, 4*ct+k, ch] = ipw[:, xcol] * w[ch, k]
    cvw = np.zeros((128, 12, 128), np.float32)
    ipx = d['in_proj_w'][:, D_INNER:D_INNER + CONV_DIM]   # [128, 320]
    for ct in range(3):
        rows = 128 if ct < 2 else 64
        cb[:rows, ct] = d['conv_b'][ct * 128:ct * 128 + rows]
        for k in range(4):
            w = d['conv_w'][ct * 128:ct * 128 + rows, k]
            cvw[:, ct * 4 + k, :rows] = ipx[:, ct * 128:ct * 128 + rows] * w[None, :]
    A = -np.exp(d['A_log'])
    sel8 = np.zeros((8, 256), np.float32)
    for m in range(256):
        sel8[m // 32, m] = 1.0
    sel4 = np.zeros((4, 128), np.float32)
    for m in range(128):
        sel4[m // 32, m] = 1.0
    ddiag = np.zeros((128, 2, 128), np.float32)
    for jt in range(2):
        for r in range(128):
            ddiag[r, jt, r] = d['D_skip'][4 * jt + r // 32]
    cmat = (np.eye(128) - np.full((128, 128), 1.0 / 128)).astype(np.float32)
    cmato = cmat @ np.diag(d['rms_w'])
    hones2 = np.zeros((16, 1024), np.float32)
    for hh in range(8):
        hones2[hh, 128 * hh:128 * (hh + 1)] = 1.0
        hones2[8 + hh, 128 * hh:128 * (hh + 1)] = 1.0
    nbones8 = np.zeros((8, 512), np.float32)
    for hl in range(2):
        for c in range(4):
            nbones8[4 * hl + c, c * 128:(c + 1) * 128] = -1.0
    utri = MBIG * (np.arange(128)[:, None] < np.arange(128)[None, :])
    eyeblk = np.zeros((128, 512), np.float32)
    for c in range(4):
        eyeblk[:, c * 128:(c + 1) * 128] = np.eye(128)
    col = lambda v: np.ascontiguousarray(v.reshape(-1, 1), dtype=np.float32)
    vals = {
        'wW1': W1.astype(BF), 'b1': col(b1v),
        'wW2': W2sb.astype(BF), 'b2': col(d['bn2_b']),
        'olng': col(d['oln_g']), 'olnb': col(d['oln_b']),
        'wq': wq_f.astype(BF),
        'wk': wk_f.astype(BF),
        'wv': wv_f.astype(BF),
        'bq': col(bq_f),
        'bkr': np.ascontiguousarray(bk_f.reshape(1, 128)).astype(BF),
        'bvr': np.ascontiguousarray(bv_f.reshape(1, 128)).astype(BF),
        'ones1r': np.ones((1, 128), BF),
        'wo': d['wo'].astype(BF), 'bo': col(d['bo']),
        'ff1w': ff1w_f.astype(BF),
        'ff1b': np.ascontiguousarray(ff1b_f.reshape(2, 128).T),
        'ff2w': ff2sb.astype(BF), 'ff2b': col(d['ff2_b']),
        'ipwz': d['in_proj_w'][:, 0:D_INNER].astype(BF),
        'ipwd': np.ascontiguousarray(
            d['in_proj_w'][:, D_INNER + CONV_DIM:]).astype(BF),
        'convb': cb, 'cvw': cvw.astype(BF),
        'dtbias': col(d['dt_bias']), 'A2': col(A),
        'outw': outsb.astype(BF),
        'sel8': sel8.astype(BF), 'sel4': sel4.astype(BF),
        'sel4T': np.ascontiguousarray(sel4.T).astype(BF),
        'hones2': _f16_as_bf(hones2), 'nbones8': _f16_as_bf(nbones8),
        'utri': _f16_as_bf(utri), 'eyeblk': _f16_as_bf(eyeblk),
        'ddiag': ddiag.astype(BF),
        'onesm128': np.full((128, 128), 1.0 / 128, BF),
        'onesm256': np.full((128, 128), 1.0 / 256, BF),
        'cmat': cmat.astype(BF), 'cmato': cmato.astype(BF),
        'eye': np.eye(128, dtype=BF),
        'onecol': np.ones((128, 1), BF),
        'epsln': np.full((128, 1), 1e-5, np.float32),
        'epsrms': np.full((128, 1), 1e-6, np.float32),
    }
    wpackf = np.zeros((128, WF_COLS), np.float32)
    wpackb = np.zeros((128, WB_COLS), BF)
    for nm, rows, cols, dt in WSPEC:
        ncols = int(np.prod(cols)) if isinstance(cols, tuple) else cols
        v = np.asarray(vals[nm]).reshape(rows, ncols)
        off = W_OFF[nm]
        if dt == "f":
            wpackf[0:rows, off:off + ncols] = v
        else:
            wpackb[0:rows, off:off + ncols] = v
    wmap = {'wpackf': wpackf, 'wpackb': wpackb}
    return wmap


def kernel(**inputs):
    if 'nc' not in _CACHE:
        _CACHE['nc'] = build_nc()
    nc = _CACHE['nc']
    wmap = _prep(inputs)
    x = np.asarray(inputs['x'], np.float32)
    in_maps = []
    for core in range(8):
        xs = x[2 * core:2 * core + 2].reshape(2, 2048, 128)
        xTv = np.ascontiguousarray(xs.transpose(2, 0, 1).reshape(128, 4096))
        m = dict(wmap)
        m['xT'] = xTv.astype(BF)
        in_maps.append(m)
    res = run_bass_kernel_spmd(nc, in_maps, core_ids=list(range(8)))
    outs = []
    for core in range(8):
        o = np.asarray(res.results[core]['out'], np.float32)   # [128, 1024]
        outs.append(np.ascontiguousarray(o.T.reshape(2, 512, 128)))
    return np.concatenate(outs, 0).astype(np.float32)


if __name__ == '__main__':
    rng = np.random.default_rng(0)
    x = rng.standard_normal((B, L, E)).astype(np.float32)
    print("built module ok")


# revision 76
# speedup vs baseline: 1.0167x; 1.0167x over previous
"""Self-contained Trainium2 kernel for nn_AssemblyArrayComponent_9019431322130.

Data-parallel over batch: 16 samples -> 8 cores x 2 samples.
Host folds (w_in @ conv1 @ bn1) and (conv2 @ bn2) into plain matmuls
(stride==kernel convs are reshapes); device runs the whole net per core:
  GEMM1+gelu -> GEMM2+gelu -> linear attention -> FF -> Mamba-2 SSD (chunked,
  Q=128) -> gated RMS -> out proj -> RMS -> LN.
Activations live as [d, t] (feature on partition, t = 2*512 tokens sample-major).

v3: the SSD decay matrices are built on the PE as fp16 hi/lo outer-product
differences accumulated in PSUM f32 (plus a +M strict-upper mask matmul so
the exp underflows to exactly 0 in the masked region), replacing the
DVE/Pool broadcast-subtract chain and all Sbc DMA broadcasts.  Y keeps two
PSUM accumulators (intra+Dskip vs state) so only the state term is scaled
by cp.  elu+1 is exp->min fused with a relu via scalar_tensor_tensor.
LN2's affine folds into ff1; D_skip folds into a diagonal matmul.
"""
import sys
sys.path.insert(0, '/opt/trn_rl_repo')
import numpy as np
import ml_dtypes

import concourse.bass as bass
import concourse.tile as tile
import concourse.mybir as mybir
from concourse import bacc, library_config
from concourse.bass_utils import run_bass_kernel_spmd

f32 = mybir.dt.float32
bf16 = mybir.dt.bfloat16
f16 = mybir.dt.float16
AF = mybir.ActivationFunctionType
OP = mybir.AluOpType
BF = ml_dtypes.bfloat16

# Guide the act-table chooser: hide exp/ln from their single-function sets so
# ln+exp phases resolve to natural_log_exp_and_others.
import concourse.hw_specs as _hw_specs
from concourse import bacc as _bacc_mod
_ORIG_TABLES = _hw_specs.get_activation_tables


def _patched_tables(arch):
    out = {}
    for name, s in _ORIG_TABLES(arch).items():
        s2 = set(s)
        if name == 'exp_and_others':
            s2.discard(AF.Exp)
        if name == 'natural_log':
            s2.discard(AF.Ln)
        out[name] = s2
    return out


_bacc_mod.get_activation_tables = _patched_tables

B, L, E = 16, 16384, 16
H = 128
NH, DH = 4, 32
FF = 256
D_STATE, HEADDIM = 32, 32
D_INNER = 2 * H
NHEADS = 8
CONV_DIM = 320
DCONV = 4
LC = 512
BN_EPS = 1e-5
Q = 128          # SSD chunk
NCH = 4          # chunks per sample
BLOC = 2         # samples per core
T = BLOC * LC    # 1024 tokens per core
MBIG = 350.0     # strict-upper mask magnitude (underflows exp to 0)


# (name, rows, cols_or_tuple, dtype-class) — "b": bf16, "h": fp16 bits stored
# in the bf16 pack (bitcast at use), "f": f32.  Ordered by first use so the
# weight-pack DMA chunks can interleave with the xT input tiles.
WSPEC = [
    ("wW1", 128, 128, "b"), ("wW2", 128, (4, 128), "b"),
    ("cmat", 128, 128, "b"), ("onesm128", 128, 128, "b"),
    ("wq", 128, 128, "b"), ("wk", 128, 128, "b"), ("wv", 128, 128, "b"),
    ("bkr", 1, 128, "b"), ("bvr", 1, 128, "b"), ("ones1r", 1, 128, "b"),
    ("onecol", 128, 1, "b"), ("sel4T", 128, 4, "b"), ("sel4", 4, 128, "b"),
    ("wo", 128, 128, "b"),
    ("ff1w", 128, 256, "b"), ("ff2w", 128, (2, 128), "b"),
    ("ipwz", 128, 256, "b"), ("ipwd", 128, 8, "b"),
    ("cvw", 128, (12, 128), "b"),
    ("eye", 128, 128, "b"), ("sel8", 8, 256, "b"),
    ("hones2", 16, 1024, "h"), ("nbones8", 8, 512, "h"),
    ("utri", 128, 128, "h"), ("eyeblk", 128, 512, "h"),
    ("ddiag", 128, (2, 128), "b"),
    ("onesm256", 128, 128, "b"), ("cmato", 128, 128, "b"),
    ("outw", 128, (2, 128), "b"),
    ("b1", 128, 1, "f"), ("b2", 128, 1, "f"),
    ("bq", 128, 1, "f"),
    ("olng", 128, 1, "f"), ("olnb", 128, 1, "f"),
    ("bo", 128, 1, "f"),
    ("ff1b", 128, 2, "f"), ("ff2b", 128, 1, "f"),
    ("convb", 128, 3, "f"),
    ("dtbias", 8, 1, "f"), ("A2", 8, 1, "f"),
    ("epsln", 128, 1, "f"), ("epsrms", 128, 1, "f"),
]
W_OFF = {}
WF_COLS = 0
WB_COLS = 0
for _nm, _r, _c, _d in WSPEC:
    _n = int(np.prod(_c)) if isinstance(_c, tuple) else _c
    if _d == "f":
        W_OFF[_nm] = WF_COLS; WF_COLS += _n
    else:
        W_OFF[_nm] = WB_COLS; WB_COLS += _n


def _ap(t_ap, offset_elems, dims):
    return bass.AP(t_ap.tensor, t_ap.offset + offset_elems, dims)


def build_nc():
    nc = bacc.Bacc('TRN2', target_bir_lowering=False, debug=False, num_devices=8)
    dram = {}

    def din(name, shape, dt):
        dram[name] = nc.dram_tensor(name, shape, dt, kind="ExternalInput")
        return dram[name]

    xT = din("xT", [128, 4096], bf16)
    wpackf = din("wpackf", [128, WF_COLS], f32)
    wpackb = din("wpackb", [128, WB_COLS], bf16)
    out_d = nc.dram_tensor("out", [128, 1024], bf16, kind="ExternalOutput")

    with tile.TileContext(nc) as tc:
        with (
            tc.tile_pool(name="wp", bufs=1) as wp,      # weights/consts
            tc.tile_pool(name="ap", bufs=1) as apool,   # persistent activations
            tc.tile_pool(name="tp", bufs=2) as tp,      # transients
            tc.tile_pool(name="pw", bufs=3, space="PSUM") as pw,   # wide psum
            tc.tile_pool(name="pb", bufs=2, space="PSUM") as pb,   # block psum
            tc.tile_pool(name="py", bufs=3, space="PSUM") as py,   # Y accum
        ):
            wpf = wp.tile([128, WF_COLS], f32, tag="wpf")
            wpb = wp.tile([128, WB_COLS], bf16, tag="wpb")
            xTs = apool.tile([128, 4096], bf16, tag="bigB", name="xTs")
            _c1 = W_OFF["ff1w"]          # end of G1/attention weights
            _c2 = W_OFF["cvw"]           # end of FF/in_proj weights
            # xT chunks + wW1 on the sync queue (GEMM1 feed); weight pack on
            # the gpsimd + scalar queues so GEMM1 is never blocked behind it.
            # one queue: HWDGE issues are a global FIFO, so order by need
            nc.sync.dma_start(wpb[:, 0:128], wpackb[:, 0:128])       # wW1
            nc.sync.dma_start(wpf[:], wpackf[:])                     # biases
            nc.sync.dma_start(xTs[:, 0:1024], xT[:, 0:1024])
            nc.sync.dma_start(wpb[:, 128:_c1], wpackb[:, 128:_c1])
            nc.sync.dma_start(xTs[:, 1024:2560], xT[:, 1024:2560])
            nc.sync.dma_start(xTs[:, 2560:4096], xT[:, 2560:4096])
            nc.sync.dma_start(wpb[:, _c1:_c2], wpackb[:, _c1:_c2])
            nc.sync.dma_start(wpb[:, _c2:], wpackb[:, _c2:])
            # per-engine warm-ups: absorb the weight-DMA waits once per engine
            wa0 = tp.tile([1, 4], f32, tag="warm", bufs=1)
            nc.vector.tensor_copy(wa0[:], wpf[0:1, 0:4])
            wb0 = tp.tile([1, 4], bf16, tag="warm", bufs=1)
            nc.vector.tensor_copy(wb0[:], wpb[0:1, 0:4])
            wg = tp.tile([2, 4], f32, tag="warm", bufs=1)
            nc.gpsimd.partition_broadcast(wg[:], wpf[0:1, 0:4])
            W = {"xT": xTs}
            for nm, rows, cols, dt in WSPEC:
                off = W_OFF[nm]
                buf = wpf if dt == "f" else wpb
                ncols = int(np.prod(cols)) if isinstance(cols, tuple) else cols
                apv = buf[0:rows, off:off + ncols]
                if dt == "h":
                    apv = apv.bitcast(f16)
                if isinstance(cols, tuple):
                    apv = apv.rearrange("p (a b) -> p a b", a=cols[0])
                W[nm] = apv

            # ------- GEMM1 + gelu, GEMM2 interleaved per sample -------
            h1 = apool.tile([128, 4096], bf16, tag="bigA", name="h1")
            h_bfs = [apool.tile([128, 512], bf16, tag=f"h_bf{s}",
                                name=f"h_bf{s}") for s in range(BLOC)]
            # all 8 G1 matmuls first (PE is in-order; don't block ready work
            # behind G2 mms that wait on Act gelus)
            g1ps = []
            for i in range(8):
                pool = pw if i % 2 == 0 else py
                ps = pool.tile([128, 512], f32,
                               tag="psw" if i % 2 == 0 else "Yp",
                               name=f"g1ps{i}")
                nc.tensor.matmul(ps[:], W["wW1"][:],
                                 W["xT"][:, bass.ts(i, 512)],
                                 start=True, stop=True)
                g1ps.append(ps)
            g2ps = []
            for grp in range(2):
                for j in range(4):
                    i = 4 * grp + j
                    nc.scalar.activation(h1[:, bass.ts(i, 512)], g1ps[i][:],
                                         AF.Gelu_apprx_tanh,
                                         bias=W["b1"][:, 0:1])
                s = grp
                ps = pw.tile([128, 512], f32, tag="psw", name=f"g2ps{s}")
                for k in range(4):
                    rhs = _ap(h1[:], s * 2048 + k, [list(h1[:].ap[0]), [4, 512]])
                    nc.tensor.matmul(ps[:], W["wW2"][:, k, :], rhs,
                                     start=(k == 0), stop=(k == 3))
                g2ps.append(ps)
                nc.scalar.activation(h_bfs[s][:], g2ps[s][:],
                                     AF.Gelu_apprx_tanh, bias=W["b2"][:, 0:1])

            # ---------------- LayerNorm helper (centered via PE) ----------
            # half-major so sample 0's chain flows without waiting sample 1;
            # sq_act computes the square on Act directly from PSUM (parallel
            # with the xcs copy on DVE).
            def layer_norm(xh, g, b, eps, tagp="ln", cmat="cmat", affine=True,
                           out_dt=bf16, sq_act=False):
                halves = []
                out = None
                if affine:
                    out = apool.tile([128, 1024], out_dt, tag=tagp + "_out")
                xcsl, eql, rstdl, xcl_ps = [], [], [], []
                for hf in range(2):
                    xc = pw.tile([128, 512], f32, tag="psw",
                                 name=f"{tagp}xc{hf}")
                    nc.tensor.matmul(xc[:], W[cmat][:], xh[hf],
                                     start=True, stop=True)
                    xcs = tp.tile([128, 512], bf16, tag="ln_xcs", bufs=2)
                    nc.vector.tensor_copy(xcs[:], xc[:])
                    xcsl.append(xcs)
                    xcl_ps.append(xc)
                for hf in range(2):
                    sq = tp.tile([128, 512], bf16, tag="ln_sq", bufs=2)
                    if sq_act:
                        nc.scalar.activation(sq[:], xcl_ps[hf][:], AF.Square)
                    else:
                        nc.vector.tensor_tensor(out=sq[:], in0=xcsl[hf][:],
                                                in1=xcsl[hf][:], op=OP.mult)
                    eq = pw.tile([128, 512], f32, tag="psw",
                                 name=f"{tagp}eq{hf}")
                    nc.tensor.matmul(eq[:], W["onesm128"][:], sq[:],
                                     start=True, stop=True)
                    eql.append(eq)
                for hf in range(2):
                    lnv = tp.tile([128, 512], f32, tag="ln_lnv", bufs=2)
                    nc.scalar.activation(lnv[:], eql[hf][:], AF.Ln,
                                         bias=eps[:, 0:1])
                    rstd = tp.tile([128, 512], bf16, tag="ln_rstd", bufs=3)
                    nc.scalar.activation(rstd[:], lnv[:], AF.Exp, scale=-0.5)
                    rstdl.append(rstd)
                for hf in range(2):
                    t2 = tp.tile([128, 512], bf16, tag=tagp + "_t2", bufs=2)
                    nc.vector.tensor_tensor(out=t2[:], in0=xcsl[hf][:],
                                            in1=rstdl[hf][:], op=OP.mult)
                    if affine:
                        nc.vector.tensor_scalar(out=out[:, bass.ts(hf, 512)],
                                                in0=t2[:], scalar1=g[:, 0:1],
                                                scalar2=b[:, 0:1],
                                                op0=OP.mult, op1=OP.add)
                    halves.append(t2)
                return out, halves

            # ---------------- attention (ln1 affine folded into wq/wk/wv) --
            _, a_hv = layer_norm([h_bfs[0][:], h_bfs[1][:]],
                                 None, None, W["epsln"], tagp="ln1",
                                 affine=False)

            # q in [dq, t]: elu+1 = min(exp(x),1) + max(x,0)
            q_bf = apool.tile([128, 1024], bf16, tag="q_bf")
            em_q = apool.tile([128, 1024], bf16, tag="em_q")
            xr_q = apool.tile([128, 1024], bf16, tag="xr_q")
            meK = apool.tile([128, 8, 128], bf16, tag="meK")
            vT = apool.tile([128, 8, 128], bf16, tag="vT")
            em_k = apool.tile([128, 1024], bf16, tag="em_k")
            xr_k = apool.tile([128, 8, 128], bf16, tag="xr_k")
            for hf in range(2):
                ps = pw.tile([128, 512], f32, tag="psw", name=f"qps{hf}")
                nc.tensor.matmul(ps[:], W["wq"][:], a_hv[hf][:],
                                 start=True, stop=True)
                psk = pw.tile([128, 512], f32, tag="psw", name=f"psk{hf}")
                psv = pw.tile([128, 512], f32, tag="psw", name=f"psv{hf}")
                for q4 in range(4):
                    nc.tensor.matmul(psk[:, bass.ts(q4, 128)],
                                     a_hv[hf][:, bass.ts(q4, 128)], W["wk"][:],
                                     start=True, stop=False)
                    nc.tensor.matmul(psk[:, bass.ts(q4, 128)],
                                     W["ones1r"][0:1, :], W["bkr"][0:1, :],
                                     start=False, stop=True)
                    nc.tensor.matmul(psv[:, bass.ts(q4, 128)],
                                     a_hv[hf][:, bass.ts(q4, 128)], W["wv"][:],
                                     start=True, stop=False)
                    nc.tensor.matmul(psv[:, bass.ts(q4, 128)],
                                     W["ones1r"][0:1, :], W["bvr"][0:1, :],
                                     start=False, stop=True)
                nc.scalar.activation(em_k[:, bass.ts(hf, 512)],
                                     psk[:], AF.Exp)
                nc.vector.tensor_scalar(
                    out=xr_k[:].rearrange("p a b -> p (a b)")[:, bass.ts(hf, 512)],
                    in0=psk[:], scalar1=0.0,
                    scalar2=None, op0=OP.max)
                if hf == 0:
                    nc.scalar.copy(
                        vT[:].rearrange("p a b -> p (a b)")[:, 0:512], psv[:])
                else:
                    nc.vector.tensor_copy(
                        vT[:].rearrange("p a b -> p (a b)")[:, 512:1024],
                        psv[:])
                nc.vector.tensor_scalar(
                    out=meK[:].rearrange("p a b -> p (a b)")[:, bass.ts(hf, 512)],
                    in0=em_k[:, bass.ts(hf, 512)], scalar1=1.0,
                    scalar2=None, op0=OP.min)
                nc.scalar.activation(em_q[:, bass.ts(hf, 512)], ps[:],
                                     AF.Exp, bias=W["bq"][:, 0:1])
                nc.vector.tensor_scalar(out=xr_q[:, bass.ts(hf, 512)],
                                        in0=ps[:], scalar1=W["bq"][:, 0:1],
                                        scalar2=0.0, op0=OP.add, op1=OP.max)
                nc.vector.scalar_tensor_tensor(
                    out=q_bf[:, bass.ts(hf, 512)],
                    in0=em_q[:, bass.ts(hf, 512)], scalar=1.0,
                    in1=xr_q[:, bass.ts(hf, 512)], op0=OP.min, op1=OP.add)

            # kv[d,e] per (b,h) stacked on partitions; ksum via ones rhs
            kv_sb, ksumM = [], []
            for s in range(BLOC):
                kvp = pb.tile([128, 32], f32, tag="psb")
                for hh in range(4):
                    for tt in range(4):
                        nc.tensor.matmul(
                            kvp[32 * hh:32 * hh + 32, :],
                            xr_k[:, 4 * s + tt, 32 * hh:32 * hh + 32],
                            vT[:, 4 * s + tt, 32 * hh:32 * hh + 32],
                            start=(tt == 0), stop=False,
                            tile_position=(0, 32 * hh))
                    for tt in range(4):
                        nc.tensor.matmul(
                            kvp[32 * hh:32 * hh + 32, :],
                            meK[:, 4 * s + tt, 32 * hh:32 * hh + 32],
                            vT[:, 4 * s + tt, 32 * hh:32 * hh + 32],
                            start=False, stop=(tt == 3),
                            tile_position=(0, 32 * hh))
                kv = apool.tile([128, 32], bf16, tag=f"kv{s}")
                nc.scalar.copy(kv[:], kvp[:])
                kv_sb.append(kv)
                ksp = pb.tile([128, 1], f32, tag="psb")
                for tt in range(4):
                    nc.tensor.matmul(ksp[:], xr_k[:, 4 * s + tt, :],
                                     W["onecol"][:],
                                     start=(tt == 0), stop=False)
                for tt in range(4):
                    nc.tensor.matmul(ksp[:], meK[:, 4 * s + tt, :],
                                     W["onecol"][:],
                                     start=False, stop=(tt == 3))
                km = apool.tile([128, 4], bf16, tag=f"ksumM{s}")
                ksp_bc = _ap(ksp[:], 0, [list(ksp[:].ap[0]), [0, 4]])
                nc.vector.tensor_tensor(out=km[:], in0=ksp_bc,
                                        in1=W["sel4T"][:], op=OP.mult)
                ksumM.append(km)

            attnf = apool.tile([128, 1024], bf16, tag="attnf")
            zrbs, zrxss, atps = {}, {}, {}
            for s in range(BLOC):
                den = pb.tile([4, 512], f32, tag="psb", name=f"den{s}")
                nc.tensor.matmul(den[:], ksumM[s][:], q_bf[:, bass.ts(s, 512)],
                                 start=True, stop=True)
                zrb = tp.tile([4, 512], bf16, tag="zrb")
                with nc.allow_low_precision(reason="z feeds bf16 matmul rhs"):
                    nc.vector.reciprocal(zrb[:], den[:])
                zrbs[s] = zrb
            for s in range(BLOC):
                atp = pw.tile([128, 512], f32, tag="psw", name=f"atp{s}")
                for hh in range(4):
                    nc.tensor.matmul(atp[32 * hh:32 * hh + 32, :],
                                     kv_sb[s][32 * hh:32 * hh + 32, :],
                                     q_bf[32 * hh:32 * hh + 32, bass.ts(s, 512)],
                                     start=True, stop=True,
                                     tile_position=(32 * hh, 32 * hh))
                atps_sb = tp.tile([128, 512], bf16, tag="atps", bufs=2)
                nc.scalar.copy(atps_sb[:], atp[:])
                atps[s] = atps_sb
            for s in range(BLOC):
                zrx = pb.tile([128, 512], f32, tag="psb", name=f"zrx{s}")
                nc.tensor.matmul(zrx[:], W["sel4"][:], zrbs[s][:],
                                 start=True, stop=True)
                nc.vector.tensor_tensor(out=attnf[:, bass.ts(s, 512)],
                                        in0=atps[s][:],
                                        in1=zrx[:], op=OP.mult)

            h2_bf = apool.tile([128, 1024], bf16, tag="h2_bf")
            h2ps = []
            for hf in range(2):
                ps = pw.tile([128, 512], f32, tag="psw", name=f"h2ps{hf}")
                nc.tensor.matmul(ps[:], W["wo"][:], attnf[:, bass.ts(hf, 512)],
                                 start=True, stop=True)
                h2ps.append(ps)
            for hf in range(2):
                nc.vector.scalar_tensor_tensor(
                    out=h2_bf[:, bass.ts(hf, 512)], in0=h2ps[hf][:],
                    scalar=W["bo"][:, 0:1], in1=h_bfs[hf][:],
                    op0=OP.add, op1=OP.add)

            # ---------------- FF (ln2 affine folded into ff1) -------------
            _, f_hv = layer_norm([h2_bf[:, 0:512], h2_bf[:, 512:1024]],
                                 None, None, W["epsln"], tagp="ln2",
                                 affine=False)
            gff = apool.tile([128, 2, 1024], bf16, tag="bigA", name="gff")
            ffps = {}
            for mt in range(2):
                for hf in range(2):
                    ps = pw.tile([128, 512], f32, tag="psw",
                                 name=f"ffps{mt}{hf}")
                    nc.tensor.matmul(ps[:], W["ff1w"][:, bass.ts(mt, 128)],
                                     f_hv[hf][:],
                                     start=True, stop=True)
                    ffps[(mt, hf)] = ps
            for mt in range(2):
                for hf in range(2):
                    nc.scalar.activation(gff[:, mt, bass.ts(hf, 512)],
                                         ffps[(mt, hf)][:],
                                         AF.Gelu_apprx_tanh,
                                         bias=W["ff1b"][:, mt:mt + 1])
            # h3 in a causally-padded layout [128, 2, 515] (3 zero cols per
            # sample) so the fused conv taps can read shifted windows
            h3p = apool.tile([128, 2, 515], bf16, tag="h3p", name="h3p")
            for hf in range(2):
                nc.gpsimd.memset(h3p[:, hf, 0:3], 0.0)
            h3ps = []
            for hf in range(2):
                ps = pw.tile([128, 512], f32, tag="psw", name=f"h3ps{hf}")
                for kt in range(2):
                    nc.tensor.matmul(ps[:], W["ff2w"][:, kt, :],
                                     gff[:, kt, bass.ts(hf, 512)],
                                     start=(kt == 0), stop=(kt == 1))
                h3ps.append(ps)
            for hf in range(2):
                nc.vector.scalar_tensor_tensor(
                    out=h3p[:, hf, 3:515], in0=h3ps[hf][:],
                    scalar=W["ff2b"][:, 0:1], in1=h2_bf[:, bass.ts(hf, 512)],
                    op0=OP.add, op1=OP.add)

            # ---------------- Mamba: in_proj ----------------
            # dt first (critical path); the depthwise conv is folded into
            # the in_proj weights (taps read the zero-padded h3 directly)
            dtraw = apool.tile([8, 1024], f32, tag="dtraw")
            dps = []
            for hf in range(2):
                ps = pw.tile([8, 512], f32, tag="psw", name=f"dps{hf}")
                nc.tensor.matmul(ps[:], W["ipwd"][:],
                                 h3p[:, hf, 3:515], start=True,
                                 stop=True)
                dps.append(ps)
            for hf in range(2):
                nc.vector.tensor_copy(dtraw[:, bass.ts(hf, 512)],
                                      dps[hf][0:8, :])

            # softplus(dtraw + dt_bias) = ln(1 + exp(x)); per-sample so the
            # s0 SSD stream launches before s1 finishes
            dt2a = apool.tile([8, 1024], f32, tag="dt2a")
            dt2bf = apool.tile([8, 1024], bf16, tag="dt2bf")
            S2 = apool.tile([8, 1024], f32, tag="S2")
            S2h = apool.tile([8, 1024], f16, tag="S2h")
            S2l = apool.tile([8, 1024], f16, tag="S2l")
            S2hl_d = nc.dram_tensor("S2hl", [16, 1024], f16)
            S2HL = apool.tile([16, 1024], f16, tag="S2HL")
            S2T4 = [apool.tile([8, 1024], f16, tag=f"S2T4_{s}",
                               name=f"S2T4_{s}")
                    for s in range(BLOC)]
            for s in range(BLOC):
                sl = bass.ts(s, 512)
                espx = tp.tile([8, 512], f32, tag="spbuf", bufs=2)
                nc.scalar.activation(espx[:], dtraw[:, sl], AF.Exp,
                                     bias=W["dtbias"][0:8, 0:1])
                nc.scalar.activation(dt2a[:, sl], espx[:], AF.Ln, bias=1.0)
                nc.gpsimd.tensor_copy(dt2bf[:, sl], dt2a[:, sl])
                for g in range(4):
                    gg = 4 * s + g
                    nc.vector.tensor_tensor_scan(
                        out=S2[:, bass.ts(gg, 128)],
                        data0=dt2a[:, bass.ts(gg, 128)],
                        data1=dt2a[:, bass.ts(gg, 128)], initial=0.0,
                        op0=OP.add, op1=OP.bypass)
                nc.vector.tensor_copy(S2h[:, sl], S2[:, sl])
                nc.vector.tensor_tensor(out=S2l[:, sl], in0=S2[:, sl],
                                        in1=S2h[:, sl], op=OP.subtract)
                nc.sync.dma_start(S2hl_d[0:8, s * 512:(s + 1) * 512],
                                  S2h[:, sl])
                nc.gpsimd.dma_start(S2hl_d[8:16, s * 512:(s + 1) * 512],
                                    S2l[:, sl])
                nc.sync.dma_start(S2HL[:, sl],
                                  S2hl_d[:, s * 512:(s + 1) * 512])
                eng = nc.gpsimd if s == 0 else nc.sync
                for hl in range(2):
                    srcp = bass.AP(S2hl_d[:].tensor, hl * 8 * 1024 + s * 512,
                                   [[128, 4], [1024, 8], [1, 128]])
                    eng.dma_start(S2T4[s][4 * hl:4 * hl + 4, :], srcp)

            # fused in_proj+depthwise-causal-conv on PE: tap k reads the
            # padded h3 shifted by k, lhsT = ipw_x-slice * diag(conv_w[:,k])
            xbcs = apool.tile([128, 6, 512], bf16, tag="xbcs")
            zgs = apool.tile([128, 2, 1024], bf16, tag="bigB", name="zgs")
            for s in range(BLOC):
                cvps = []
                for ct in range(3):
                    rows = 128 if ct < 2 else 64
                    pool_ = pw if s == 0 else py
                    cps_ = pool_.tile([128, 512], f32,
                                      tag="psw" if s == 0 else "Yp",
                                      name=f"cvps{s}{ct}")
                    for k in range(4):
                        nc.tensor.matmul(
                            cps_[0:rows, :],
                            W["cvw"][:, 4 * ct + k, 0:rows],
                            h3p[:, s, k:512 + k],
                            start=(k == 0), stop=(k == 3))
                    cvps.append(cps_)
                for ct in range(3):
                    rows = 128 if ct < 2 else 64
                    nc.scalar.activation(xbcs[0:rows, 3 * s + ct, :],
                                         cvps[ct][0:rows, :], AF.Silu,
                                         bias=W["convb"][0:rows, ct:ct + 1])
                # z-gate pair for this half between the conv silus so the
                # zps PSUM slots recycle before the s1 z matmuls issue
                mt = s
                zps = []
                for hf in range(2):
                    ps = pw.tile([128, 512], f32, tag="psw",
                                 name=f"zps{mt}{hf}")
                    nc.tensor.matmul(ps[:], W["ipwz"][:, bass.ts(mt, 128)],
                                     h3p[:, hf, 3:515],
                                     start=True, stop=True)
                    zps.append(ps)
                for hf in range(2):
                    nc.scalar.activation(zgs[:, mt, bass.ts(hf, 512)],
                                         zps[hf][:], AF.Silu)

            # cp = exp(A*S2); wend = exp(A*(S_end - S2)) — after the silu
            # phase (the dr/LT stream does not need them; only the state
            # chain does), so the conv PSUM slots recycle sooner
            cp8 = apool.tile([8, 1024], bf16, tag="cp8")
            wend_bf = apool.tile([8, 1024], bf16, tag="wend_bf")
            for s in range(BLOC):
                sl = bass.ts(s, 512)
                nc.scalar.activation(cp8[:, sl], S2[:, sl], AF.Exp,
                                     scale=W["A2"][0:8, 0:1])
                wl = tp.tile([8, 512], f32, tag="wl", bufs=2)
                send_ap = _ap(S2[:], s * 512 + 127,
                              [list(S2[:].ap[0]), [128, 4], [0, 128]])
                nc.vector.tensor_tensor(
                    out=wl[:].rearrange("p (c j) -> p c j", c=4),
                    in0=send_ap,
                    in1=S2[:, sl].rearrange("p (c j) -> p c j", c=4),
                    op=OP.subtract)
                nc.scalar.activation(wend_bf[:, sl], wl[:], AF.Exp,
                                     scale=W["A2"][0:8, 0:1])
            # wendT [128, 8*8] (cols g*8+h); 4 transposes per PSUM + 1 copy
            wendT = apool.tile([128, 64], bf16, tag="wendT")
            for gb in range(2):
                ptw = pb.tile([128, 32], bf16, tag="psb", name=f"ptw{gb}")
                for k in range(4):
                    g = 4 * gb + k
                    nc.tensor.transpose(ptw[:, bass.ts(k, 8)],
                                        wend_bf[:, bass.ts(g, 128)],
                                        W["eye"][0:8, 0:8])
                nc.vector.tensor_copy(wendT[:, bass.ts(gb, 32)], ptw[:])

            # dt broadcast + xdt
            xdt = apool.tile([128, 6, 512], bf16, tag="bigD", name="xdt")
            dxs = {}
            for s in range(BLOC):
                for jt in range(2):
                    dx = pw.tile([128, 512], f32, tag="psw", name=f"dx{s}{jt}")
                    nc.tensor.matmul(dx[:], W["sel8"][:, bass.ts(jt, 128)],
                                     dt2bf[:, bass.ts(s, 512)],
                                     start=True, stop=True)
                    dxs[(s, jt)] = dx
            for s in range(BLOC):
                for jt in range(2):
                    nc.vector.tensor_tensor(out=xdt[:, 3 * s + jt, :],
                                            in0=xbcs[:, 3 * s + jt, :],
                                            in1=dxs[(s, jt)][:], op=OP.mult)

            # xdtT [t_local, (tb, ch256)] per sample: batched transposes
            xdtT = [apool.tile([128, 4, 256], bf16, tag=f"xdtT{s}", name=f"xdtT{s}")
                    for s in range(BLOC)]
            for s in range(BLOC):
                ptps = []
                for jt in range(2):
                    ptp = pb.tile([128, 512], bf16, tag="psb",
                                  name=f"ptp{s}{jt}")
                    for tb in range(4):
                        nc.tensor.transpose(
                            ptp[:, bass.ts(tb, 128)],
                            xdt[:, 3 * s + jt, bass.ts(tb, 128)], W["eye"][:])
                    ptps.append(ptp)
                for jt in range(2):
                    out_ap = _ap(xdtT[s][:], jt * 128,
                                 [list(xdtT[s][:].ap[0]), [256, 4], [1, 128]])
                    if s == 1:
                        nc.scalar.copy(
                            out_ap,
                            ptps[jt][:].rearrange("p (a b) -> p a b", a=4))
                    else:
                        nc.vector.tensor_copy(
                            out_ap,
                            ptps[jt][:].rearrange("p (a b) -> p a b", a=4))

            # xdtw = xdtT * wend (per-head, free-broadcast over p)
            xdtw = [apool.tile([128, 4, 256], bf16, tag=f"xdtw{s}", name=f"xdtw{s}")
                    for s in range(BLOC)]
            for s in range(BLOC):
                for tb in range(4):
                    wap = _ap(wendT[:], (4 * s + tb) * 8,
                              [list(wendT[:].ap[0]), [1, 8], [0, 32]])
                    eng = nc.gpsimd if tb % 2 else nc.vector
                    eng.tensor_tensor(
                        out=xdtw[s][:, tb, :].rearrange("p (h q) -> p h q", h=8),
                        in0=xdtT[s][:, tb, :].rearrange("p (h q) -> p h q", h=8),
                        in1=wap, op=OP.mult)

            # BT [t_local, (tb, n32)] per sample
            BT = [apool.tile([128, 4, 32], bf16, tag=f"BT{s}", name=f"BT{s}")
                  for s in range(BLOC)]
            for s in range(BLOC):
                for tbp in range(2):
                    pts = []
                    for k in range(2):
                        tb = 2 * tbp + k
                        pt = pb.tile([128, 128], bf16, tag="psb",
                                     name=f"btp{s}{tb}")
                        nc.tensor.transpose(
                            pt[:], xbcs[:, 3 * s + 2, bass.ts(tb, 128)],
                            W["eye"][:])
                        pts.append(pt)
                    for k in range(2):
                        tb = 2 * tbp + k
                        if tb < 3:
                            nc.vector.tensor_copy(BT[s][:, tb, :],
                                                  pts[k][:, 0:32])

            # chunk states first: the recurrence (DVE) runs while the PE
            # builds the decay matrices.  states[s][c] = state at START of
            # chunk c (bf16 [32, 256]).
            states = [[None] * NCH for _ in range(BLOC)]
            aendSB = {}
            for s in range(BLOC):
                st0 = apool.tile([32, 256], bf16, tag=f"st_{s}_0")
                nc.vector.memset(st0[:], 0.0)
                states[s][0] = st0
            for c in range(NCH - 1):
                for s in range(BLOC):
                    aend8 = tp.tile([8, 32], bf16, tag="aend8", bufs=2)
                    aap = _ap(cp8[:], s * 512 + c * 128 + 127,
                              [list(cp8[:].ap[0]), [0, 32]])
                    nc.vector.tensor_copy(aend8[:], aap)
                    aendB = pb.tile([32, 256], f32, tag="psb",
                                    name=f"aendB{s}{c}")
                    nc.tensor.matmul(aendB[:], aend8[:], W["sel8"][:],
                                     start=True, stop=True,
                                     tile_position=(0, 0))
                    aendSB[(s, c)] = aendB
                for s in range(BLOC):
                    Tp = pb.tile([32, 256], f32, tag="psb", name=f"Tp{s}{c}")
                    nc.tensor.matmul(Tp[:], BT[s][:, c, :], xdtw[s][:, c, :],
                                     start=True, stop=True)
                    st_tmp = tp.tile([32, 256], bf16, tag="st_tmp", bufs=2)
                    nc.vector.tensor_tensor(out=st_tmp[:],
                                            in0=states[s][c][:],
                                            in1=aendSB[(s, c)][:], op=OP.mult)
                    st2 = apool.tile([32, 256], bf16, tag=f"st_{s}_{c + 1}")
                    nc.vector.tensor_tensor(out=st2[:], in0=st_tmp[:],
                                            in1=Tp[:], op=OP.add)
                    states[s][c + 1] = st2

            # Bm/Cm at partition base 0 [32, 512] per sample
            Bm_sb = [apool.tile([32, 512], bf16, tag=f"Bm{s}", name=f"Bm{s}")
                     for s in range(BLOC)]
            Cm_sb = [apool.tile([32, 512], bf16, tag=f"Cm{s}", name=f"Cm{s}")
                     for s in range(BLOC)]
            for s in range(BLOC):
                nc.gpsimd.tensor_copy(Bm_sb[s][:], xbcs[0:32, 3 * s + 2, :])
                nc.gpsimd.tensor_copy(Cm_sb[s][:], xbcs[32:64, 3 * s + 2, :])

            # G = Bm^T Cm per chunk, unmasked (upper region killed by LT=0)
            GTms = [apool.tile([128, 512], bf16, tag=f"GTms{s}",
                               name=f"GTms{s}") for s in range(BLOC)]
            for s in range(BLOC):
                gp = pw.tile([128, 512], f32, tag="psw", name=f"gp{s}")
                for c in range(4):
                    nc.tensor.matmul(gp[:, bass.ts(c, 128)],
                                     Bm_sb[s][:, bass.ts(c, 128)],
                                     Cm_sb[s][:, bass.ts(c, 128)],
                                     start=True, stop=True)
                nc.vector.tensor_copy(GTms[s][:], gp[:])

            # ---- decay matrices via PE outer products (fp16 hi/lo) ----
            # dr[t',(c,j)] = S2_j - S2_t' + M*(j<t');  LT = exp(A_h * dr);
            # MT = LT * G.  Emitted interleaved with the Yp phases and the
            # per-sample tails so each sample drains while the next builds.
            MT = [[apool.tile([128, 512], bf16, tag=f"MT{s}_{hh}",
                              name=f"MT{s}_{hh}")
                   for hh in range(NHEADS)] for s in range(BLOC)]
            y2 = apool.tile([128, 6, 512], bf16, tag="bigD", name="y2")
            yn = apool.tile([128, 6, 512], bf16, tag="bigC", name="yn")
            yfin = apool.tile([128, 1024], bf16, tag="yfin")
            yzs, eqs, rstds, cpxs = {}, {}, {}, {}

            def emit_MT(s, hh):
                dr = pw.tile([128, 512], f32, tag="psw", name=f"dr{s}{hh}")
                # term1: +S2_j broadcast down columns (hi+lo stacked)
                nc.tensor.matmul(dr[:], W["hones2"][:, bass.ts(hh, 128)],
                                 S2HL[:, bass.ts(s, 512)],
                                 start=True, stop=False)
                # term2: -S2_t' per chunk (hi+lo stacked on (hl,c))
                nc.tensor.matmul(dr[:], S2T4[s][:, bass.ts(hh, 128)],
                                 W["nbones8"][:],
                                 start=False, stop=False)
                # +M on the strict upper triangle of each chunk block
                nc.tensor.matmul(dr[:], W["utri"][:], W["eyeblk"][:],
                                 start=False, stop=True)
                LT = tp.tile([128, 512], bf16, tag="LT", bufs=3)
                nc.scalar.activation(LT[:], dr[:], AF.Exp,
                                     scale=-float(hh + 1))
                eng = nc.vector if s == 0 else nc.gpsimd
                eng.tensor_tensor(out=MT[s][hh][:], in0=LT[:],
                                  in1=GTms[s][:], op=OP.mult)

            def emit_Yp(s, jt):
                # YpI = intra + D_skip*xs;  YpS = state terms.
                # y2 = (YpS * cp) + YpI; then the gated-RMS square stage.
                cx = pw.tile([128, 512], f32, tag="psw", name=f"cx{s}{jt}")
                nc.tensor.matmul(cx[:], W["sel8"][:, bass.ts(jt, 128)],
                                 cp8[:, bass.ts(s, 512)],
                                 start=True, stop=True)
                cxs = tp.tile([128, 512], bf16, tag="cpx_sb", bufs=2)
                nc.scalar.copy(cxs[:], cx[:])
                cpxs[(s, jt)] = cxs
                YpI = py.tile([128, 512], f32, tag="Yp", name=f"YpI{s}{jt}")
                YpS = py.tile([128, 512], f32, tag="Yp", name=f"YpS{s}{jt}")
                # chunk 0 has no state yet: zero-fill via 0-weight matmul
                # (hones2 row 0 is zero past col 128)
                nc.tensor.matmul(YpS[:, 0:128],
                                 W["hones2"][0:1, 128:256],
                                 W["hones2"][0:1, 0:128],
                                 start=True, stop=True)
                for c in range(NCH):
                    for hq in range(4):
                        hh = 4 * jt + hq
                        nc.tensor.matmul(
                            YpI[32 * hq:32 * hq + 32, bass.ts(c, 128)],
                            xdtT[s][:, c, 32 * hh:32 * hh + 32],
                            MT[s][hh][:, bass.ts(c, 128)],
                            start=True, stop=False,
                            tile_position=(0, 32 * hq))
                        if c > 0:
                            nc.tensor.matmul(
                                YpS[32 * hq:32 * hq + 32, bass.ts(c, 128)],
                                states[s][c][:, 32 * hh:32 * hh + 32],
                                Cm_sb[s][:, bass.ts(c, 128)],
                                start=True, stop=True,
                                tile_position=(0, 32 * hq))
                # D_skip fold: YpI += Ddiag @ xs
                nc.tensor.matmul(YpI[:], W["ddiag"][:, jt, :],
                                 xbcs[:, 3 * s + jt, :],
                                 start=False, stop=True)
                t1 = tp.tile([128, 512], bf16, tag="t1", bufs=2)
                nc.vector.tensor_tensor(out=t1[:], in0=YpS[:],
                                        in1=cpxs[(s, jt)][:], op=OP.mult)
                nc.vector.tensor_tensor(out=y2[:, 3 * s + jt, :],
                                        in0=t1[:], in1=YpI[:], op=OP.add)
                yz = tp.tile([128, 512], bf16, tag="yz", name=f"yz{s}{jt}")
                nc.vector.tensor_tensor(out=yz[:], in0=y2[:, 3 * s + jt, :],
                                        in1=zgs[:, jt, bass.ts(s, 512)],
                                        op=OP.mult)
                yzs[(s, jt)] = yz
                sqz = tp.tile([128, 512], bf16, tag="sqz", name=f"sqz{s}{jt}")
                nc.vector.tensor_tensor(out=sqz[:], in0=yz[:],
                                        in1=yz[:], op=OP.mult)
                if jt == 0:
                    eqs[s] = pb.tile([128, 512], f32, tag="psb",
                                     name=f"eqmn{s}")
                nc.tensor.matmul(eqs[s][:], W["onesm256"][:], sqz[:],
                                 start=(jt == 0), stop=(jt == 1))

            def emit_mnorm(s, mh):
                # half-split (256 cols) so the out-proj quarters start sooner
                sl = slice(mh * 256, mh * 256 + 256)
                lnv = tp.tile([128, 256], f32, tag="mn_lnv", bufs=2)
                nc.scalar.activation(lnv[:], eqs[s][:, sl], AF.Ln,
                                     bias=W["epsrms"][:, 0:1])
                rstd = tp.tile([128, 256], bf16, tag="mn_rstd", bufs=2)
                nc.scalar.activation(rstd[:], lnv[:], AF.Exp, scale=-0.5)
                # mnorm_w folded into out_w's contraction dim (host-side)
                for jt in range(2):
                    nc.vector.tensor_tensor(
                        out=yn[:, 3 * s + jt, sl], in0=yzs[(s, jt)][:, sl],
                        in1=rstd[:], op=OP.mult)

            def emit_oln(s, hq):
                # out-proj + residual + rms_w + oln centering fused on PE:
                # xc = cmato @ (outw @ yn + h3); quarter-width for latency
                xc = pw.tile([128, 256], f32, tag="psw", name=f"xcol{s}{hq}")
                for kt in range(2):
                    nc.tensor.matmul(xc[:], W["outw"][:, kt, :],
                                     _ap(yn[:], (3 * s + kt) * 512 + hq * 256,
                                         [list(yn[:].ap[0]), [1, 256]]),
                                     start=(kt == 0), stop=False)
                nc.tensor.matmul(xc[:], W["cmato"][:],
                                 h3p[:, s, 3 + hq * 256:3 + hq * 256 + 256],
                                 start=False, stop=True)
                xcs = tp.tile([128, 256], bf16, tag="ol_xcs", bufs=3)
                nc.vector.tensor_copy(xcs[:], xc[:])
                sq = tp.tile([128, 256], bf16, tag="ol_sq", bufs=3)
                nc.vector.tensor_tensor(out=sq[:], in0=xcs[:],
                                        in1=xcs[:], op=OP.mult)
                eq = pw.tile([128, 256], f32, tag="psw", name=f"eqol{s}{hq}")
                nc.tensor.matmul(eq[:], W["onesm128"][:], sq[:],
                                 start=True, stop=True)
                lnv = tp.tile([128, 256], f32, tag="ol_lnv", bufs=3)
                nc.scalar.activation(lnv[:], eq[:], AF.Ln,
                                     bias=W["epsln"][:, 0:1])
                rstd = tp.tile([128, 256], bf16, tag="ol_rstd", bufs=4)
                nc.scalar.activation(rstd[:], lnv[:], AF.Exp, scale=-0.5)
                t2 = tp.tile([128, 256], bf16, tag="oln_t2", bufs=3)
                nc.vector.tensor_tensor(out=t2[:], in0=xcs[:],
                                        in1=rstd[:], op=OP.mult)
                off = s * 512 + hq * 256
                nc.vector.tensor_scalar(out=yfin[:, off:off + 256],
                                        in0=t2[:], scalar1=W["olng"][:, 0:1],
                                        scalar2=W["olnb"][:, 0:1],
                                        op0=OP.mult, op1=OP.add)
                nc.sync.dma_start(out_d[:, off:off + 256],
                                  yfin[:, off:off + 256])

            for hh in range(NHEADS):
                emit_MT(0, hh)
            emit_Yp(0, 0)
            for hh in range(4):
                emit_MT(1, hh)
            emit_Yp(0, 1)
            emit_mnorm(0, 0)
            emit_mnorm(0, 1)
            for hh in range(4, NHEADS):
                emit_MT(1, hh)
            emit_Yp(1, 0)
            emit_Yp(1, 1)
            emit_oln(0, 0)
            emit_oln(0, 1)
            emit_mnorm(1, 0)
            emit_oln(1, 0)
            emit_mnorm(1, 1)
            emit_oln(1, 1)

    nc.compile()
    return nc


# ---------------- host side ----------------
_CACHE = {}


def _f16_as_bf(v):
    """fp16 bit pattern viewed as bfloat16 for the bf16 weight pack."""
    return np.ascontiguousarray(v.astype(np.float16)).view(np.uint16).view(BF)


def _prep(inputs):
    d = {k: np.asarray(v, np.float32) for k, v in inputs.items()}
    inv = 1.0 / np.sqrt(1.0 + BN_EPS)
    W1 = np.einsum('ei,oik->keo', d['w_in'], d['conv1_w']).reshape(128, H)
    b1v = np.einsum('i,oik->o', d['b_in'], d['conv1_w'])
    s1 = d['bn1_g'] * inv
    W1 = W1 * s1[None, :]
    b1v = b1v * s1 + d['bn1_b']
    W2 = np.transpose(d['conv2_w'], (2, 1, 0)) * (d['bn2_g'] * inv)[None, None, :]
    W2sb = np.ascontiguousarray(np.transpose(W2, (1, 0, 2)))          # [i,k,o]
    # ln1 affine folded into wq/wk/wv
    wq_f = d['ln1_g'][:, None] * d['wq']
    wk_f = d['ln1_g'][:, None] * d['wk']
    wv_f = d['ln1_g'][:, None] * d['wv']
    bq_f = d['ln1_b'] @ d['wq']
    bk_f = d['ln1_b'] @ d['wk']
    bv_f = d['ln1_b'] @ d['wv']
    # ln2 affine folded into ff1
    ff1w_f = d['ln2_g'][:, None] * d['ff1_w']
    ff1b_f = d['ff1_b'] + d['ln2_b'] @ d['ff1_w']
    ff2sb = np.ascontiguousarray(d['ff2_w'].reshape(2, 128, 128).transpose(1, 0, 2))
    _cm = (np.eye(128) - np.full((128, 128), 1.0 / 128)) @ np.diag(d['rms_w'])
    _ow2 = (d['mnorm_w'][:, None] * d['out_w']) @ _cm
    outsb = np.ascontiguousarray(_ow2.reshape(2, 128, 128).transpose(1, 0, 2))
    cb = np.zeros((128, 3), np.float32)
    # conv folded into in_proj: cvw[:, 4*ct+k, ch] = ipw[:, xcol] * w[ch, k]
    cvw = np.zeros((128, 12, 128), np.float32)
    ipx = d['in_proj_w'][:, D_INNER:D_INNER + CONV_DIM]   # [128, 320]
    for ct in range(3):
        rows = 128 if ct < 2 else 64
        cb[:rows, ct] = d['conv_b'][ct * 128:ct * 128 + rows]
        for k in range(4):
            w = d['conv_w'][ct * 128:ct * 128 + rows, k]
            cvw[:, ct * 4 + k, :rows] = ipx[:, ct * 128:ct * 128 + rows] * w[None, :]
    A = -np.exp(d['A_log'])
    sel8 = np.zeros((8, 256), np.float32)
    for m in range(256):
        sel8[m // 32, m] = 1.0
    sel4 = np.zeros((4, 128), np.float32)
    for m in range(128):
        sel4[m // 32, m] = 1.0
    ddiag = np.zeros((128, 2, 128), np.float32)
    for jt in range(2):
        for r in range(128):
            ddiag[r, jt, r] = d['D_skip'][4 * jt + r // 32]
    cmat = (np.eye(128) - np.full((128, 128), 1.0 / 128)).astype(np.float32)
    cmato = cmat @ np.diag(d['rms_w'])
    hones2 = np.zeros((16, 1024), np.float32)
    for hh in range(8):
        hones2[hh, 128 * hh:128 * (hh + 1)] = 1.0
        hones2[8 + hh, 128 * hh:128 * (hh + 1)] = 1.0
    nbones8 = np.zeros((8, 512), np.float32)
    for hl in range(2):
        for c in range(4):
            nbones8[4 * hl + c, c * 128:(c + 1) * 128] = -1.0
    utri = MBIG * (np.arange(128)[:, None] < np.arange(128)[None, :])
    eyeblk = np.zeros((128, 512), np.float32)
    for c in range(4):
        eyeblk[:, c * 128:(c + 1) * 128] = np.eye(128)
    col = lambda v: np.ascontiguousarray(v.reshape(-1, 1), dtype=np.float32)
    vals = {
        'wW1': W1.astype(BF), 'b1': col(b1v),
        'wW2': W2sb.astype(BF), 'b2': col(d['bn2_b']),
        'olng': col(d['oln_g']), 'olnb': col(d['oln_b']),
        'wq': wq_f.astype(BF),
        'wk': wk_f.astype(BF),
        'wv': wv_f.astype(BF),
        'bq': col(bq_f),
        'bkr': np.ascontiguousarray(bk_f.reshape(1, 128)).astype(BF),
        'bvr': np.ascontiguousarray(bv_f.reshape(1, 128)).astype(BF),
        'ones1r': np.ones((1, 128), BF),
        'wo': d['wo'].astype(BF), 'bo': col(d['bo']),
        'ff1w': ff1w_f.astype(BF),
        'ff1b': np.ascontiguousarray(ff1b_f.reshape(2, 128).T),
        'ff2w': ff2sb.astype(BF), 'ff2b': col(d['ff2_b']),
        'ipwz': d['in_proj_w'][:, 0:D_INNER].astype(BF),
        'ipwd': np.ascontiguousarray(
            d['in_proj_w'][:, D_INNER + CONV_DIM:]).astype(BF),
        'convb': cb, 'cvw': cvw.astype(BF),
        'dtbias': col(d['dt_bias']), 'A2': col(A),
        'outw': outsb.astype(BF),
        'sel8': sel8.astype(BF), 'sel4': sel4.astype(BF),
        'sel4T': np.ascontiguousarray(sel4.T).astype(BF),
        'hones2': _f16_as_bf(hones2), 'nbones8': _f16_as_bf(nbones8),
        'utri': _f16_as_bf(utri), 'eyeblk': _f16_as_bf(eyeblk),
        'ddiag': ddiag.astype(BF),
        'onesm128': np.full((128, 128), 1.0 / 128, BF),
        'onesm256': np.full((128, 128), 1.0 / 256, BF),
        'cmat': cmat.astype(BF), 'cmato': cmato.astype(BF),
        'eye': np.eye(128, dtype=BF),
        'onecol': np.ones((128, 1), BF),
        'epsln': np.full((128, 1), 1e-5, np.float32),
        'epsrms': np.full((128, 1), 1e-6, np.float32),
    }
    wpackf = np.zeros((128, WF_COLS), np.float32)
    wpackb = np.zeros((128, WB_COLS), BF)
    for nm, rows, cols, dt in WSPEC:
        ncols = int(np.prod(cols)) if isinstance(cols, tuple) else cols
        v = np.asarray(vals[nm]).reshape(rows, ncols)
        off = W_OFF[nm]
        if dt == "f":
            wpackf[0:rows, off:off + ncols] = v
        else:
            wpackb[0:rows, off:off + ncols] = v
    wmap = {'wpackf': wpackf, 'wpackb': wpackb}
    return wmap


def kernel(**inputs):
    if 'nc' not in _CACHE:
        _CACHE['nc'] = build_nc()
    nc = _CACHE['nc']
    wmap = _prep(inputs)
    x = np.asarray(inputs['x'], np.float32)
    in_maps = []
    for core in range(8):
        xs = x[2 * core:2 * core + 2].reshape(2, 2048, 128)
        xTv = np.ascontiguousarray(xs.transpose(2, 0, 1).reshape(128, 4096))
        m = dict(wmap)
        m['xT'] = xTv.astype(BF)
        in_maps.append(m)
    res = run_bass_kernel_spmd(nc, in_maps, core_ids=list(range(8)))
    outs = []
    for core in range(8):
        o = np.asarray(res.results[core]['out'], np.float32)   # [128, 1024]
        outs.append(np.ascontiguousarray(o.T.reshape(2, 512, 128)))
    return np.concatenate(outs, 0).astype(np.float32)


if __name__ == '__main__':
    rng = np.random.default_rng(0)
    x = rng.standard_normal((B, L, E)).astype(np.float32)
    print("built module ok")
